# revision 63
# baseline (speedup 1.0000x reference)
"""Trainium2 Bass kernel for nn_CustomABlock (MDTA transformer block).

Per-core layout: one batch image [C=256, N=4096(=64x64)] per NeuronCore,
data-parallel over B=8 across 8 cores, all params replicated.

Engine plan (per core):
  PE   : qkv matmul (f32r), 2 dwconv taps (diag matmul), q/k transposes,
         gram (attn logits), attn@v, proj, mlp1, mlp2
  DVE  : 6 dwconv taps (scalar_tensor_tensor FMA, bf16), residual adds,
         reciprocals, row-max reduces, x1 bf16 copy
  ACT  : PSUM drains, l2norm squares (accum), exp (softmax), gelu+bias
  GPSIMD: 1 dwconv tap, identity build
"""

import numpy as np
import ml_dtypes

BF16 = ml_dtypes.bfloat16

C = 256          # dim
N = 4096         # 64*64
H = W = 64
NH = 8           # heads
CH = 32          # channels per head
HID = 307        # mlp hidden
NB_QKV = 6       # qkv channel blocks of 128
NT = 8           # n tiles of 512
TS = 512

# tap index t = (dy+1)*3 + (dx+1)
PE_TAPS = [(0, 0), (-1, 0), (1, 0), (0, -1), (0, 1)]  # PE diag matmuls into PSUM
MERGE_TAP = (1, 1)                  # DVE STT: tap + PSUM drain in one op
DVE_TAPS = [(-1, -1), (-1, 1), (1, -1)]   # DVE tensor_scalar + tensor_tensor
PE2_TAPS = [(-1, -1), (1, -1)]      # extra PE corner taps for late blocks 1/3

_CACHE = {}


def _build_bass():
    import concourse.bass as bass
    from concourse import bacc
    from concourse import mybir
    from concourse.tile import TileContext
    from concourse.masks import make_identity

    # Steer the act-table-load pass: hide Exp/Ln from every set except
    # natural_log_exp_and_others so all transcendentals (norm ln/exp +
    # softmax exp) share one table load instead of ping-ponging sets.
    # Set order (= act_func_set_id) is preserved; the chosen set really
    # does contain both functions, so the loads stay correct.
    if not getattr(bacc, "_act_tables_patched", False):
        _orig_tables = bacc.get_activation_tables
        AF_ = mybir.ActivationFunctionType

        def _patched_tables(arch):
            tabs = {k: set(v) for k, v in _orig_tables(arch).items()}
            for name, fns in tabs.items():
                if name != "natural_log_exp_and_others":
                    fns.discard(AF_.Exp)
                    fns.discard(AF_.Ln)
            return tabs

        bacc.get_activation_tables = _patched_tables
        bacc._act_tables_patched = True

    dt = mybir.dt
    f32 = dt.float32
    f32r = dt.float32r
    bf16 = dt.bfloat16
    AF = mybir.ActivationFunctionType
    OP = mybir.AluOpType

    nc = bacc.Bacc("TRN2")

    f8 = dt.float8e4

    # ---- DRAM I/O (per-core) ----
    xb_d = nc.dram_tensor("xb", [128, 2, N], bf16, kind="ExternalInput")
    xf8_d = nc.dram_tensor("xf8", [128, 2, N], f8, kind="ExternalInput")
    wqkv8_d = nc.dram_tensor("wqkv8", [128, 2, 3 * C], f8, kind="ExternalInput")
    wdiag_d = nc.dram_tensor("wdiag", [128, len(PE_TAPS), 4, 128], bf16,
                             kind="ExternalInput")
    wdiag2_d = nc.dram_tensor("wdiag2", [128, 2, 2, 128], bf16,
                              kind="ExternalInput")
    hmask_d = nc.dram_tensor("hmask", [128, 128], bf16, kind="ExternalInput")
    wdw_d = nc.dram_tensor("wdw", [128, NB_QKV * 9 * 2], f32, kind="ExternalInput")
    wproj_d = nc.dram_tensor("wproj8", [128, 2, C], f8, kind="ExternalInput")
    wm1_d = nc.dram_tensor("wm1T", [128, 2, HID], bf16, kind="ExternalInput")
    wm2_d = nc.dram_tensor("wm28", [128, 3, C], f8, kind="ExternalInput")
    b1_d = nc.dram_tensor("b1", [128, 3], f32, kind="ExternalInput")
    b2_d = nc.dram_tensor("b2", [128, 2], f32, kind="ExternalInput")
    lntv_d = nc.dram_tensor("lntv", [128, 2], f32, kind="ExternalInput")
    out_d = nc.dram_tensor("out", [128, 2, N], f32, kind="ExternalOutput")

    with TileContext(nc) as tc:
        with (
            tc.tile_pool(name="wpool", bufs=1) as wpool,
            tc.tile_pool(name="xpool", bufs=1) as xpool,
            tc.tile_pool(name="qkvp", bufs=4) as qkvp,       # qkv_s blocks / ys reuse
            tc.tile_pool(name="dwqk", bufs=4) as dwqk_p,     # dw q/k blocks
            tc.tile_pool(name="dwv", bufs=2) as dwv_p,       # x1b tiles
            tc.tile_pool(name="qt", bufs=1) as qt_p,
            tc.tile_pool(name="scr", bufs=2) as scr_p,
            tc.tile_pool(name="small", bufs=10) as small_p,
            tc.tile_pool(name="dg", bufs=2) as dg_p,
            tc.tile_pool(name="bt", bufs=18) as b_p,
            tc.tile_pool(name="attn", bufs=1) as atn_p,
            tc.tile_pool(name="ysp", bufs=1) as ysp,
            tc.tile_pool(name="outp", bufs=2) as out_p,
            tc.tile_pool(name="apool", bufs=2) as a_p,
            tc.tile_pool(name="pbig", bufs=2, space="PSUM") as pbig,
            tc.tile_pool(name="pdw", bufs=2, space="PSUM") as pdw,
            tc.tile_pool(name="psml", bufs=2, space="PSUM") as psml,
        ):
            # dummy Ln on an always-ready memset tile: makes the FIRST
            # act-table load the natural_log_exp set (which also covers
            # copy/square); emitted before any other ACT-stream work
            zz = small_p.tile([128, 1], f32, tag="zz")
            nc.vector.memset(zz, 1.0)
            dmy = small_p.tile([128, 1], f32, tag="dmy")
            nc.scalar.activation(out=dmy, in_=zz, func=AF.Ln)

            # ---- load weights & x (critical path first) ----
            xr = xpool.tile([128, 2, N], bf16)
            xf8 = xpool.tile([128, 2, N], f8)
            wqkv_s = wpool.tile([128, 2, 3 * C], f8)
            for kb in range(2):
                nc.sync.dma_start(out=wqkv_s[:, kb, :], in_=wqkv8_d[:, kb, :])
            # x fp8 chunks t-ordered so the first qkv matmuls start early;
            # split across the two HWDGE rings (SP + ACT) to halve the
            # serial startup latency
            for t in range(4):
                for kb in range(2):
                    nc.sync.dma_start(
                        out=xf8[:, kb, t * 1024:(t + 1) * 1024],
                        in_=xf8_d[:, kb, t * 1024:(t + 1) * 1024])
            wdiag_s = wpool.tile([128, len(PE_TAPS), 4, 128], bf16)
            nc.scalar.dma_start(out=wdiag_s, in_=wdiag_d[:, :, :, :])
            wdiag2_s = wpool.tile([128, 2, 2, 128], bf16)
            nc.scalar.dma_start(out=wdiag2_s, in_=wdiag2_d[:, :, :, :])
            wdw_s = wpool.tile([128, NB_QKV * 9 * 2], f32)
            nc.scalar.dma_start(out=wdw_s, in_=wdw_d[:, :])
            # xr (bf16 residual base) is only needed by the tail
            for kb in range(2):
                nc.sync.dma_start(out=xr[:, kb, :], in_=xb_d[:, kb, :])
            hmask_s = wpool.tile([128, 128], bf16)
            nc.scalar.dma_start(out=hmask_s[:, :], in_=hmask_d[:, :])
            wproj_s = wpool.tile([128, 2, C], f8)
            nc.scalar.dma_start(out=wproj_s, in_=wproj_d[:, :, :])
            wm1_s = wpool.tile([128, 2, HID], bf16)
            nc.scalar.dma_start(out=wm1_s, in_=wm1_d[:, :, :])
            wm2_s = wpool.tile([128, 3, C], f8)
            nc.scalar.dma_start(out=wm2_s, in_=wm2_d[:, :, :])
            b1_s = wpool.tile([128, 3], f32)
            nc.scalar.dma_start(out=b1_s, in_=b1_d[:, :])
            b2_s = wpool.tile([128, 2], f32)
            nc.scalar.dma_start(out=b2_s, in_=b2_d[:, :])
            lntv_s = wpool.tile([128, 2], f32)
            nc.scalar.dma_start(out=lntv_s, in_=lntv_d[:, :])

            ident = wpool.tile([128, 128], bf16)
            make_identity(nc, ident)

            # ---- per-block pipeline ----
            dw_tiles = [None] * NB_QKV
            qT_s = qt_p.tile([128, 32, C], bf16, tag="qT")
            kT_s = qt_p.tile([128, 32, C], bf16, tag="kT")
            attn8 = atn_p.tile([128, 2, N], f8, tag="attn")
            rs_v = [None, None]
            At_v = [None, None]
            rnq_v = [None, None]

            qkv_v = [None, None]   # v_lin tiles for the av-fold
            qkv_tiles = {}
            rhs_ops = {}

            DR = mybir.MatmulPerfMode.DoubleRow

            def qkv_phase(ob):
                # qkv = W_qkv @ x: fp8 DoubleRow folds the K=256 contraction
                # into one matmul (weights pre-scaled x16; drain undoes it)
                qkv_t = qkvp.tile([128, N], bf16, tag="qkv", name=f"qkv{ob}")
                for t in range(4):
                    ps = pbig.tile([128, 1024], f32, tag="pbig", name="ps")
                    for h in range(2):
                        nc.tensor.matmul(
                            ps[:, h * TS:(h + 1) * TS],
                            lhsT=wqkv_s[:, :, ob * 128:(ob + 1) * 128],
                            rhs=xf8[:, :, t * 1024 + h * TS:
                                    t * 1024 + (h + 1) * TS],
                            perf_mode=DR, start=True, stop=True,
                        )
                    nc.scalar.mul(qkv_t[:, t * 1024:(t + 1) * 1024], ps,
                                  1.0 / 16.0)
                qkv_tiles[ob] = qkv_t
                if ob >= 4:
                    qkv_v[ob - 4] = qkv_t

            def tap_phase(ob):
                # dwconv: PE diag taps (flat shifts) -> PSUM[128,512],
                # merge/corner taps + wrap fixups on DVE, then the l2 norm
                qkv_t = qkv_tiles[ob]
                dw_t = dwqk_p.tile([128, N], bf16, tag="dwqk", name=f"dw{ob}")
                dw_tiles[ob] = dw_t
                dw3 = dw_t.rearrange("p (y x) -> p y x", y=H)
                qk3 = qkv_t.rearrange("p (y x) -> p y x", y=H)
                dym, dxm = MERGE_TAP
                wm = wdw_s[:, ob * 9 + (dym + 1) * 3 + dxm + 1:
                           ob * 9 + (dym + 1) * 3 + dxm + 2]
                w01 = wdw_s[:, 54 + ob * 9 + 5:54 + ob * 9 + 6]
                # blocks 1/3 run late: move 2 corner taps to PE to shorten
                # their serial DVE chain (wrap-fixup columns handled below)
                pe_corner = ob in (1, 3)
                for t8 in range(8):
                    pd = pdw.tile([128, TS], f32, tag="pdw", name="pd")
                    pd3 = pd.rearrange("p (y x) -> p y x", y=8)
                    c0 = t8 * TS
                    ops = []
                    for ti, (dy, dx) in enumerate(PE_TAPS):
                        s = dy * 64 + dx
                        a = max(c0, -s)
                        b = min(c0 + TS, N - max(0, s))
                        if a < b:
                            ops.append(((0, ti), s, a, b))
                    if pe_corner:
                        for tj, (dy, dx) in enumerate(PE2_TAPS):
                            s = dy * 64 + dx
                            a = max(c0, -s)
                            b = min(c0 + TS, N - max(0, s))
                            if a < b:
                                ops.append(((1, tj), s, a, b))
                    for j, ((bank, ti), s, a, b) in enumerate(ops):
                        lhs = (wdiag_s[:, ti, ob, :] if bank == 0 else
                               wdiag2_s[:, ti, (ob - 1) // 2, :])
                        nc.tensor.matmul(
                            pd[:, a - c0:b - c0],
                            lhsT=lhs,
                            rhs=qkv_t[:, a + s:b + s],
                            start=(j == 0), stop=(j == len(ops) - 1),
                        )
                    yt = t8 * 8
                    # merge tap (1,1): dw = w*qkv[y+1,x+1] + psum (drains)
                    ya, yb = yt, min(yt + 8, 63)
                    nc.vector.scalar_tensor_tensor(
                        out=dw3[:, ya:yb, 0:63],
                        in0=qk3[:, ya + 1:yb + 1, 1:64],
                        scalar=wm,
                        in1=pd3[:, 0:yb - yt, 0:63],
                        op0=OP.mult, op1=OP.add,
                    )
                    # x=63 col: drain PSUM minus tap(0,1) row-wrap
                    nc.vector.scalar_tensor_tensor(
                        out=dw3[:, yt:yb, 63:64],
                        in0=qk3[:, yt + 1:yb + 1, 0:1],
                        scalar=w01, in1=pd3[:, 0:yb - yt, 63:64],
                        op0=OP.mult, op1=OP.add,
                    )
                    if t8 == 7:
                        nc.scalar.copy(out=dw3[:, 63:64, :],
                                       in_=pd3[:, 7:8, :])
                # x=0 col: subtract tap(0,-1) row-wrap (whole block, in place)
                w0m = wdw_s[:, 54 + ob * 9 + 3:54 + ob * 9 + 4]
                nc.vector.scalar_tensor_tensor(
                    out=dw3[:, 1:64, 0:1], in0=qk3[:, 0:63, 63:64],
                    scalar=w0m, in1=dw3[:, 1:64, 0:1],
                    op0=OP.mult, op1=OP.add,
                )

                if pe_corner:
                    # wrap fixups for the PE corner taps (subtract the
                    # spurious x-wrap column contributions)
                    # (-1,-1) s=-65: out(y,0) wrongly read (y-2,63)
                    wn = wdw_s[:, 54 + ob * 9 + 0:54 + ob * 9 + 1]
                    nc.vector.scalar_tensor_tensor(
                        out=dw3[:, 2:64, 0:1], in0=qk3[:, 0:62, 63:64],
                        scalar=wn, in1=dw3[:, 2:64, 0:1],
                        op0=OP.mult, op1=OP.add)
                    # (1,-1) s=+63: out(y,0) wrongly read (y,63)
                    wn6 = wdw_s[:, 54 + ob * 9 + 6:54 + ob * 9 + 7]
                    nc.vector.scalar_tensor_tensor(
                        out=dw3[:, 0:64, 0:1], in0=qk3[:, 0:64, 63:64],
                        scalar=wn6, in1=dw3[:, 0:64, 0:1],
                        op0=OP.mult, op1=OP.add)
                # remaining corner taps: tensor_scalar (4x) + tensor_tensor
                for (dy, dx) in ([(-1, 1)] if pe_corner else DVE_TAPS):
                    ti = (dy + 1) * 3 + (dx + 1)
                    w_ap = wdw_s[:, ob * 9 + ti:ob * 9 + ti + 1]
                    y0, y1 = max(0, -dy), 64 - max(0, dy)
                    x0, x1 = max(0, -dx), 64 - max(0, dx)
                    sc_t = scr_p.tile([128, N], bf16, tag="sqscr",
                                      name=f"scr{ob}_{ti}")
                    sc3 = sc_t.rearrange("p (y x) -> p y x", y=H)
                    nc.vector.tensor_scalar_mul(
                        sc3[:, y0:y1, x0:x1],
                        qk3[:, y0 + dy:y1 + dy, x0 + dx:x1 + dx], w_ap)
                    nc.vector.tensor_tensor(
                        out=dw3[:, y0:y1, x0:x1], in0=dw3[:, y0:y1, x0:x1],
                        in1=sc3[:, y0:y1, x0:x1], op=OP.add)

                # l2 norm: ssq -> rn = exp(-0.5*ln(ssq) [+ ln(T)]), all in
                # the natural_log_exp ACT table set
                sq = scr_p.tile([128, N], bf16, tag="sqscr")
                ssq = small_p.tile([128, 1], f32, tag="ssq")
                nc.scalar.activation(out=sq, in_=dw_t, func=AF.Square,
                                     accum_out=ssq)
                lg = small_p.tile([128, 1], f32, tag="lg")
                nc.scalar.activation(out=lg, in_=ssq, func=AF.Ln)
                if ob < 2:
                    # q: T/|q| applied later as the softmax-exp scale — the
                    # transposes below don't wait on the norm chain
                    rn = small_p.tile([128, 1], f32, tag=f"rnq{ob}")
                    nc.scalar.activation(out=rn, in_=lg, func=AF.Exp,
                                         scale=-0.5,
                                         bias=lntv_s[:, ob:ob + 1])
                    rnq_v[ob] = rn
                    rhs_ops[ob] = ident
                else:
                    # k: scale must be in kT before the gram — fold into the
                    # transpose matmul via D = diag(rn)
                    rn = small_p.tile([128, 1], f32, tag="rn")
                    nc.scalar.activation(out=rn, in_=lg, func=AF.Exp,
                                         scale=-0.5)
                    D_t = dg_p.tile([128, 128], bf16, tag="D")
                    nc.vector.tensor_scalar_mul(D_t, ident, rn)
                    rhs_ops[ob] = D_t

            def tp_phase(ob):
                dw_t = dw_tiles[ob]
                rhs_op = rhs_ops[ob]
                dst = qT_s if ob < 2 else kT_s
                cof = (ob % 2) * 128
                for g in range(8):
                    tp_t = psml.tile([128, 512], f32, tag="tp")
                    for i in range(4):
                        nb = g * 4 + i
                        # regular matmul: out = dw_chunk.T @ rhs_op — a
                        # transpose that (for k) applies the row scale
                        # (transpose-mode ignores the operand values)
                        nc.tensor.matmul(
                            tp_t[:, i * 128:(i + 1) * 128],
                            lhsT=dw_t[:, nb * 128:(nb + 1) * 128],
                            rhs=rhs_op, start=True, stop=True)
                    nc.scalar.copy(
                        out=dst[:, g * 4:g * 4 + 4, cof:cof + 128],
                        in_=tp_t.rearrange("p (a b) -> p a b", a=4))

            def do_gram(g):
                # raw gram (q unnormalized; k pre-scaled); softmax as single
                # full-row ops with T*rn_q folded into the exp scale and
                # cross-head entries killed by a block-diagonal mask
                pg = psml.tile([128, 128], f32, tag="tp")
                co = g * 128
                for nb in range(32):
                    nc.tensor.matmul(
                        pg,
                        lhsT=qT_s[:, nb, co:co + 128],
                        rhs=kT_s[:, nb, co:co + 128],
                        start=(nb == 0), stop=(nb == 31),
                    )
                rnq = rnq_v[g]
                mx = small_p.tile([128, 1], f32, tag="mx")
                nc.vector.tensor_reduce(out=mx, in_=pg,
                                        axis=mybir.AxisListType.X, op=OP.max)
                ngm = small_p.tile([128, 1], f32, tag="ngm")
                nc.vector.tensor_scalar(out=ngm, in0=mx, scalar1=rnq,
                                        scalar2=-1.0, op0=OP.mult,
                                        op1=OP.mult)
                A_t = a_p.tile([128, 128], bf16, tag="A")
                nc.scalar.activation(out=A_t, in_=pg, func=AF.Exp,
                                     scale=rnq, bias=ngm)
                nc.vector.tensor_tensor(out=A_t, in0=A_t, in1=hmask_s,
                                        op=OP.mult)
                sm = small_p.tile([128, 1], f32, tag="sm")
                nc.vector.tensor_reduce(out=sm, in_=A_t,
                                        axis=mybir.AxisListType.X, op=OP.add)
                rs = small_p.tile([128, 1], f32, tag="rs")
                nc.vector.reciprocal(rs, sm)
                rs_v[g] = rs
                pa = psml.tile([128, 128], bf16, tag="tp")
                nc.tensor.transpose(pa, A_t, ident)
                At = a_p.tile([128, 128], bf16, tag="At")
                nc.scalar.copy(out=At, in_=pa)
                At_v[g] = At

            ALL_TAPS = [(0, 0), (-1, -1), (-1, 0), (-1, 1), (0, -1),
                        (0, 1), (1, -1), (1, 0), (1, 1)]

            def do_av(g):
                # dwconv folded into attention: out = sum_t shift_t(B_t @ v)
                # with B_t[d,c] = At[d,c] * w_dw[v-chan d, tap t]
                Bts = []
                for t9, (dy, dx) in enumerate(ALL_TAPS):
                    Bt = b_p.tile([128, 128], bf16, tag="Bt",
                                  name=f"B{g}_{t9}")
                    wcol = (4 + g) * 9 + (dy + 1) * 3 + (dx + 1)
                    nc.vector.tensor_scalar_mul(
                        Bt, At_v[g], wdw_s[:, wcol:wcol + 1])
                    Bts.append(Bt)
                v3 = qkv_v[g].rearrange("p (y x) -> p y x", y=H)
                for t8 in range(8):
                    yt = t8 * 8
                    pv = pdw.tile([128, TS], f32, tag="pdw", name="pv")
                    pv3 = pv.rearrange("p (y x) -> p y x", y=8)
                    ops = []
                    for t9, (dy, dx) in enumerate(ALL_TAPS):
                        ya, yb = max(yt, -dy), min(yt + 8, 64 - dy)
                        xa, xb = max(0, -dx), 64 - max(0, dx)
                        if ya < yb:
                            ops.append((t9, dy, dx, ya, yb, xa, xb))
                    for j, (t9, dy, dx, ya, yb, xa, xb) in enumerate(ops):
                        nc.tensor.matmul(
                            pv3[:, ya - yt:yb - yt, xa:xb],
                            lhsT=Bts[t9],
                            rhs=v3[:, ya + dy:yb + dy, xa + dx:xb + dx],
                            start=(j == 0), stop=(j == len(ops) - 1))
                    nc.scalar.mul(attn8[:, g, yt * 64:(yt + 8) * 64],
                                  pv, rs_v[g])

            # software-pipelined emission: each engine's stream executes in
            # order, so later-phase PE work (transposes/gram/av) is emitted
            # only once enough independent PE work precedes it to cover the
            # DVE/ACT chains it waits on
            qkv_phase(2)
            tap_phase(2)
            qkv_phase(0)
            tap_phase(0)
            qkv_phase(4)
            qkv_phase(3)
            tap_phase(3)
            tp_phase(2)
            tp_phase(0)
            do_gram(0)
            qkv_phase(1)
            tap_phase(1)
            qkv_phase(5)
            tp_phase(3)
            do_av(0)
            tp_phase(1)
            do_gram(1)
            do_av(1)

            # ---- streamed tail ----
            # residuals are folded into PSUM via identity matmuls, so the
            # per-tile chain is PE -> ACT -> PE -> ACT (no DVE hops)
            x1b = [dwv_p.tile([128, N], bf16, tag="dwv", name=f"x1b{i}")
                   for i in range(2)]
            ys_t = ysp.tile([128, 3, N], f8, tag="ysf8", name="ys")
            # ones-row at hidden index 307 (kb2-local row 51): the mlp2
            # weight row there holds 16*b2, folding the bias into the matmul
            nc.vector.memset(ys_t[:, 2, :], 1.0)

            def proj_phase(t):
                sl = slice(t * 1024, (t + 1) * 1024)
                for ob in range(2):
                    pp = pbig.tile([128, 1024], f32, tag="pbig", name="pp")
                    for h in range(2):
                        nc.tensor.matmul(
                            pp[:, h * TS:(h + 1) * TS],
                            lhsT=wproj_s[:, :, ob * 128:(ob + 1) * 128],
                            rhs=attn8[:, :, t * 1024 + h * TS:
                                      t * 1024 + (h + 1) * TS],
                            perf_mode=DR, start=True, stop=True)
                    nc.vector.scalar_tensor_tensor(
                        out=x1b[ob][:, sl], in0=pp, scalar=1.0 / 16.0,
                        in1=xr[:, ob, sl], op0=OP.mult, op1=OP.add)

            def mlp1_phase(t):
                sl = slice(t * 1024, (t + 1) * 1024)
                for mb in range(3):
                    rows = 128 if mb < 2 else HID - 256
                    pm = pbig.tile([128, 1024], f32, tag="pbig", name="pm")
                    for h in range(2):
                        for kb in range(2):
                            nc.tensor.matmul(
                                pm[:rows, h * TS:(h + 1) * TS],
                                lhsT=wm1_s[:, kb, mb * 128:mb * 128 + rows],
                                rhs=x1b[kb][:, t * 1024 + h * TS:
                                            t * 1024 + (h + 1) * TS],
                                start=(kb == 0), stop=(kb == 1))
                    nc.scalar.activation(
                        out=ys_t[:rows, mb, sl],
                        in_=pm[:rows, :], func=AF.Gelu_apprx_tanh,
                        bias=b1_s[:rows, mb:mb + 1])

            def mlp2_phase(t):
                sl = slice(t * 1024, (t + 1) * 1024)
                for ob in range(2):
                    pm2 = pbig.tile([128, 1024], f32, tag="pbig", name="pm2")
                    for h in range(2):
                        hs = slice(t * 1024 + h * TS, t * 1024 + (h + 1) * TS)
                        nc.tensor.matmul(
                            pm2[:, h * TS:(h + 1) * TS],
                            lhsT=wm2_s[:, 0:2, ob * 128:(ob + 1) * 128],
                            rhs=ys_t[:, 0:2, hs],
                            perf_mode=DR, start=True, stop=False)
                        nc.tensor.matmul(
                            pm2[:, h * TS:(h + 1) * TS],
                            lhsT=wm2_s[:52, 2, ob * 128:(ob + 1) * 128],
                            rhs=ys_t[:52, 2, hs],
                            start=False, stop=True)
                    ot = out_p.tile([128, 1024], f32, tag="ot",
                                    name=f"ot{t}_{ob}")
                    nc.vector.scalar_tensor_tensor(
                        out=ot, in0=pm2,
                        scalar=1.0 / 16.0, in1=x1b[ob][:, sl],
                        op0=OP.mult, op1=OP.add)
                    nc.sync.dma_start(out=out_d[:, ob, sl], in_=ot)

            # pipelined emission: every PE group's ACT dependency is covered
            # by the preceding PE group
            proj_phase(0)
            proj_phase(1)
            mlp1_phase(0)
            mlp1_phase(1)
            mlp2_phase(0)
            proj_phase(2)
            mlp1_phase(2)
            mlp2_phase(1)
            proj_phase(3)
            mlp1_phase(3)
            mlp2_phase(2)
            mlp2_phase(3)

    return nc


def _prep_shared(w_qkv, w_dw, temperature, w_proj, w_mlp1, b_mlp1, w_mlp2, b_mlp2):
    f32 = np.float32
    shared = {}
    F8 = ml_dtypes.float8_e4m3
    shared["wqkv8"] = np.ascontiguousarray(
        (w_qkv.T * 16.0).reshape(2, 128, 3 * C).transpose(1, 0, 2)).astype(F8)
    wd = np.zeros((128, len(PE_TAPS), 4, 128), BF16)
    for ti, (dy, dx) in enumerate(PE_TAPS):
        for cb in range(4):
            w = w_dw[cb * 128:(cb + 1) * 128, 0, dy + 1, dx + 1].astype(f32)
            wd[:, ti, cb, :] = np.diag(w).astype(BF16)
    shared["wdiag"] = wd
    wd2 = np.zeros((128, 2, 2, 128), BF16)
    for tj, (dy, dx) in enumerate(PE2_TAPS):
        for bi, cb in enumerate((1, 3)):
            w = w_dw[cb * 128:(cb + 1) * 128, 0, dy + 1, dx + 1].astype(f32)
            wd2[:, tj, bi, :] = np.diag(w).astype(BF16)
    shared["wdiag2"] = wd2
    hm = np.zeros((128, 128), f32)
    for h4 in range(4):
        hm[h4 * 32:(h4 + 1) * 32, h4 * 32:(h4 + 1) * 32] = 1.0
    shared["hmask"] = hm.astype(BF16)
    wt = np.zeros((128, NB_QKV * 9 * 2), f32)
    for cb in range(NB_QKV):
        for t in range(9):
            wt[:, cb * 9 + t] = w_dw[cb * 128:(cb + 1) * 128, 0, t // 3, t % 3]
    wt[:, 54:] = -wt[:, :54]
    shared["wdw"] = wt
    shared["wproj8"] = np.ascontiguousarray(
        (w_proj.T * 16.0).reshape(2, 128, C).transpose(1, 0, 2)).astype(F8)
    shared["wm1T"] = np.ascontiguousarray(
        w_mlp1.T.reshape(2, 128, HID).transpose(1, 0, 2)).astype(BF16)
    w2 = np.zeros((384, C), f32)
    w2[:HID] = w_mlp2.T * 16.0
    w2[307] = b_mlp2 * 16.0     # ones-row in ys folds the bias in
    shared["wm28"] = np.ascontiguousarray(
        w2.reshape(3, 128, C).transpose(1, 0, 2)).astype(F8)
    b1 = np.zeros((384,), f32)
    b1[:HID] = b_mlp1
    shared["b1"] = np.ascontiguousarray(b1.reshape(3, 128).T)
    shared["b2"] = np.ascontiguousarray(b_mlp2.astype(f32).reshape(2, 128).T)
    t = temperature.reshape(NH).astype(f32)
    tv = np.zeros((128, 2), f32)
    for g in range(2):
        tv[:, g] = np.repeat(t[g * 4:(g + 1) * 4], 32)
    shared["lntv"] = np.log(np.maximum(tv, 1e-30)).astype(f32)
    return shared


def kernel(x, w_qkv, w_dw, temperature, w_proj, w_mlp1, b_mlp1, w_mlp2, b_mlp2,
           _trace=False):
    from concourse.bass_utils import run_bass_kernel_spmd

    if "nc" not in _CACHE:
        nc = _build_bass()
        nc.finalize()
        _CACHE["nc"] = nc
    nc = _CACHE["nc"]

    x = np.asarray(x, np.float32)
    B = x.shape[0]
    shared = _prep_shared(
        np.asarray(w_qkv, np.float32), np.asarray(w_dw, np.float32),
        np.asarray(temperature, np.float32), np.asarray(w_proj, np.float32),
        np.asarray(w_mlp1, np.float32), np.asarray(b_mlp1, np.float32),
        np.asarray(w_mlp2, np.float32), np.asarray(b_mlp2, np.float32))

    in_maps = []
    for i in range(B):
        m = dict(shared)
        xi = np.ascontiguousarray(x[i].reshape(2, 128, N).transpose(1, 0, 2))
        m["xb"] = xi.astype(BF16)
        m["xf8"] = xi.astype(ml_dtypes.float8_e4m3)
        in_maps.append(m)

    res = run_bass_kernel_spmd(nc, in_maps, core_ids=list(range(B)),
                               trace=_trace)
    outs = np.stack([
        r["out"].transpose(1, 0, 2).reshape(C, H, W) for r in res.results
    ])
    if _trace:
        _CACHE["last_exec_ns"] = res.exec_time_ns
        _CACHE["last_profile"] = res.profile_json
    return outs



# revision 65
# speedup vs baseline: 1.0256x; 1.0256x over previous
"""Trainium2 Bass kernel for nn_CustomABlock (MDTA transformer block).

Per-core layout: one batch image [C=256, N=4096(=64x64)] per NeuronCore,
data-parallel over B=8 across 8 cores, all params replicated.

Engine plan (per core):
  PE   : qkv matmul (f32r), 2 dwconv taps (diag matmul), q/k transposes,
         gram (attn logits), attn@v, proj, mlp1, mlp2
  DVE  : 6 dwconv taps (scalar_tensor_tensor FMA, bf16), residual adds,
         reciprocals, row-max reduces, x1 bf16 copy
  ACT  : PSUM drains, l2norm squares (accum), exp (softmax), gelu+bias
  GPSIMD: 1 dwconv tap, identity build
"""

import numpy as np
import ml_dtypes

BF16 = ml_dtypes.bfloat16

C = 256          # dim
N = 4096         # 64*64
H = W = 64
NH = 8           # heads
CH = 32          # channels per head
HID = 307        # mlp hidden
NB_QKV = 6       # qkv channel blocks of 128
NT = 8           # n tiles of 512
TS = 512

# tap index t = (dy+1)*3 + (dx+1)
PE_TAPS = [(0, 0), (-1, 0), (1, 0), (0, -1), (0, 1)]  # PE diag matmuls into PSUM
MERGE_TAP = (1, 1)                  # DVE STT: tap + PSUM drain in one op
DVE_TAPS = [(-1, -1), (-1, 1), (1, -1)]   # DVE tensor_scalar + tensor_tensor
PE2_TAPS = [(-1, -1), (1, -1)]      # extra PE corner taps for late blocks 1/3

_CACHE = {}


def _build_bass():
    import concourse.bass as bass
    from concourse import bacc
    from concourse import mybir
    from concourse.tile import TileContext
    from concourse.masks import make_identity

    # Steer the act-table-load pass: hide Exp/Ln from every set except
    # natural_log_exp_and_others so all transcendentals (norm ln/exp +
    # softmax exp) share one table load instead of ping-ponging sets.
    # Set order (= act_func_set_id) is preserved; the chosen set really
    # does contain both functions, so the loads stay correct.
    if not getattr(bacc, "_act_tables_patched", False):
        _orig_tables = bacc.get_activation_tables
        AF_ = mybir.ActivationFunctionType

        def _patched_tables(arch):
            tabs = {k: set(v) for k, v in _orig_tables(arch).items()}
            for name, fns in tabs.items():
                if name != "natural_log_exp_and_others":
                    fns.discard(AF_.Exp)
                    fns.discard(AF_.Ln)
            return tabs

        bacc.get_activation_tables = _patched_tables
        bacc._act_tables_patched = True

    dt = mybir.dt
    f32 = dt.float32
    f32r = dt.float32r
    bf16 = dt.bfloat16
    AF = mybir.ActivationFunctionType
    OP = mybir.AluOpType

    nc = bacc.Bacc("TRN2")

    f8 = dt.float8e4

    # ---- DRAM I/O (per-core) ----
    xb_d = nc.dram_tensor("xb", [128, 2, N], bf16, kind="ExternalInput")
    xf8_d = nc.dram_tensor("xf8", [128, 2, N], f8, kind="ExternalInput")
    wqkv8_d = nc.dram_tensor("wqkv8", [128, 2, 3 * C], f8, kind="ExternalInput")
    wdiag_d = nc.dram_tensor("wdiag", [128, len(PE_TAPS), 4, 128], bf16,
                             kind="ExternalInput")
    wdiag2_d = nc.dram_tensor("wdiag2", [128, 2, 2, 128], bf16,
                              kind="ExternalInput")
    hmask_d = nc.dram_tensor("hmask", [128, 128], bf16, kind="ExternalInput")
    wdw_d = nc.dram_tensor("wdw", [128, NB_QKV * 9 * 2], f32, kind="ExternalInput")
    wproj_d = nc.dram_tensor("wproj8", [128, 2, C], f8, kind="ExternalInput")
    wm1_d = nc.dram_tensor("wm1T", [128, 2, HID], bf16, kind="ExternalInput")
    wm2_d = nc.dram_tensor("wm28", [128, 3, C], f8, kind="ExternalInput")
    b1_d = nc.dram_tensor("b1", [128, 3], f32, kind="ExternalInput")
    lntv_d = nc.dram_tensor("lntv", [128, 2], f32, kind="ExternalInput")
    out_d = nc.dram_tensor("out", [128, 2, N], f32, kind="ExternalOutput")

    with TileContext(nc) as tc:
        with (
            tc.tile_pool(name="wpool", bufs=1) as wpool,
            tc.tile_pool(name="xpool", bufs=1) as xpool,
            tc.tile_pool(name="qkvp", bufs=4) as qkvp,       # qkv_s blocks / ys reuse
            tc.tile_pool(name="dwqk", bufs=4) as dwqk_p,     # dw q/k blocks
            tc.tile_pool(name="dwv", bufs=2) as dwv_p,       # x1b tiles
            tc.tile_pool(name="qt", bufs=1) as qt_p,
            tc.tile_pool(name="scr", bufs=2) as scr_p,
            tc.tile_pool(name="small", bufs=10) as small_p,
            tc.tile_pool(name="dg", bufs=2) as dg_p,
            tc.tile_pool(name="bt", bufs=18) as b_p,
            tc.tile_pool(name="attn", bufs=1) as atn_p,
            tc.tile_pool(name="ysp", bufs=1) as ysp,
            tc.tile_pool(name="outp", bufs=2) as out_p,
            tc.tile_pool(name="apool", bufs=2) as a_p,
            tc.tile_pool(name="pbig", bufs=2, space="PSUM") as pbig,
            tc.tile_pool(name="pdw", bufs=2, space="PSUM") as pdw,
            tc.tile_pool(name="psml", bufs=2, space="PSUM") as psml,
        ):
            # dummy Ln on an always-ready memset tile: makes the FIRST
            # act-table load the natural_log_exp set (which also covers
            # copy/square); emitted before any other ACT-stream work
            zz = small_p.tile([128, 1], f32, tag="zz")
            nc.vector.memset(zz, 1.0)
            dmy = small_p.tile([128, 1], f32, tag="dmy")
            nc.scalar.activation(out=dmy, in_=zz, func=AF.Ln)

            # ---- load weights & x (critical path first) ----
            xr = xpool.tile([128, 2, N], bf16)
            xf8 = xpool.tile([128, 2, N], f8)
            wqkv_s = wpool.tile([128, 2, 3 * C], f8)
            for kb in range(2):
                nc.sync.dma_start(out=wqkv_s[:, kb, :], in_=wqkv8_d[:, kb, :])
            # x fp8 chunks t-ordered so the first qkv matmuls start early;
            # split across the two HWDGE rings (SP + ACT) to halve the
            # serial startup latency
            for t in range(4):
                for kb in range(2):
                    nc.sync.dma_start(
                        out=xf8[:, kb, t * 1024:(t + 1) * 1024],
                        in_=xf8_d[:, kb, t * 1024:(t + 1) * 1024])
            wdiag_s = wpool.tile([128, len(PE_TAPS), 4, 128], bf16)
            nc.scalar.dma_start(out=wdiag_s, in_=wdiag_d[:, :, :, :])
            wdiag2_s = wpool.tile([128, 2, 2, 128], bf16)
            nc.scalar.dma_start(out=wdiag2_s, in_=wdiag2_d[:, :, :, :])
            wdw_s = wpool.tile([128, NB_QKV * 9 * 2], f32)
            nc.scalar.dma_start(out=wdw_s, in_=wdw_d[:, :])
            # xr (bf16 residual base) is only needed by the tail
            for kb in range(2):
                nc.sync.dma_start(out=xr[:, kb, :], in_=xb_d[:, kb, :])
            # tail-phase weights go via the idle GPSIMD's SWDGE ring so the
            # ACT stream isn't blocked by DMA-issue instructions
            hmask_s = wpool.tile([128, 128], bf16)
            nc.gpsimd.dma_start(out=hmask_s[:, :], in_=hmask_d[:, :])
            wproj_s = wpool.tile([128, 2, C], f8)
            nc.gpsimd.dma_start(out=wproj_s, in_=wproj_d[:, :, :])
            wm1_s = wpool.tile([128, 2, HID], bf16)
            nc.gpsimd.dma_start(out=wm1_s, in_=wm1_d[:, :, :])
            wm2_s = wpool.tile([128, 3, C], f8)
            nc.gpsimd.dma_start(out=wm2_s, in_=wm2_d[:, :, :])
            b1_s = wpool.tile([128, 3], f32)
            nc.gpsimd.dma_start(out=b1_s, in_=b1_d[:, :])
            lntv_s = wpool.tile([128, 2], f32)
            nc.scalar.dma_start(out=lntv_s, in_=lntv_d[:, :])

            ident = wpool.tile([128, 128], bf16)
            make_identity(nc, ident)

            # ---- per-block pipeline ----
            dw_tiles = [None] * NB_QKV
            qT_s = qt_p.tile([128, 32, C], bf16, tag="qT")
            kT_s = qt_p.tile([128, 32, C], bf16, tag="kT")
            attn8 = atn_p.tile([128, 2, N], f8, tag="attn")
            rs_v = [None, None]
            At_v = [None, None]
            rnq_v = [None, None]

            qkv_v = [None, None]   # v_lin tiles for the av-fold
            qkv_tiles = {}
            rhs_ops = {}

            DR = mybir.MatmulPerfMode.DoubleRow

            def qkv_phase(ob):
                # qkv = W_qkv @ x: fp8 DoubleRow folds the K=256 contraction
                # into one matmul (weights pre-scaled x16; drain undoes it)
                qkv_t = qkvp.tile([128, N], bf16, tag="qkv", name=f"qkv{ob}")
                for t in range(4):
                    ps = pbig.tile([128, 1024], f32, tag="pbig", name="ps")
                    for h in range(2):
                        nc.tensor.matmul(
                            ps[:, h * TS:(h + 1) * TS],
                            lhsT=wqkv_s[:, :, ob * 128:(ob + 1) * 128],
                            rhs=xf8[:, :, t * 1024 + h * TS:
                                    t * 1024 + (h + 1) * TS],
                            perf_mode=DR, start=True, stop=True,
                        )
                    nc.scalar.mul(qkv_t[:, t * 1024:(t + 1) * 1024], ps,
                                  1.0 / 16.0)
                qkv_tiles[ob] = qkv_t
                if ob >= 4:
                    qkv_v[ob - 4] = qkv_t

            def tap_phase(ob):
                # dwconv: PE diag taps (flat shifts) -> PSUM[128,512],
                # merge/corner taps + wrap fixups on DVE, then the l2 norm
                qkv_t = qkv_tiles[ob]
                dw_t = dwqk_p.tile([128, N], bf16, tag="dwqk", name=f"dw{ob}")
                dw_tiles[ob] = dw_t
                dw3 = dw_t.rearrange("p (y x) -> p y x", y=H)
                qk3 = qkv_t.rearrange("p (y x) -> p y x", y=H)
                dym, dxm = MERGE_TAP
                wm = wdw_s[:, ob * 9 + (dym + 1) * 3 + dxm + 1:
                           ob * 9 + (dym + 1) * 3 + dxm + 2]
                w01 = wdw_s[:, 54 + ob * 9 + 5:54 + ob * 9 + 6]
                # blocks 1/3 run late: move 2 corner taps to PE to shorten
                # their serial DVE chain (wrap-fixup columns handled below)
                pe_corner = ob in (1, 3)
                for t8 in range(8):
                    pd = pdw.tile([128, TS], f32, tag="pdw", name="pd")
                    pd3 = pd.rearrange("p (y x) -> p y x", y=8)
                    c0 = t8 * TS
                    ops = []
                    for ti, (dy, dx) in enumerate(PE_TAPS):
                        s = dy * 64 + dx
                        a = max(c0, -s)
                        b = min(c0 + TS, N - max(0, s))
                        if a < b:
                            ops.append(((0, ti), s, a, b))
                    if pe_corner:
                        for tj, (dy, dx) in enumerate(PE2_TAPS):
                            s = dy * 64 + dx
                            a = max(c0, -s)
                            b = min(c0 + TS, N - max(0, s))
                            if a < b:
                                ops.append(((1, tj), s, a, b))
                    for j, ((bank, ti), s, a, b) in enumerate(ops):
                        lhs = (wdiag_s[:, ti, ob, :] if bank == 0 else
                               wdiag2_s[:, ti, (ob - 1) // 2, :])
                        nc.tensor.matmul(
                            pd[:, a - c0:b - c0],
                            lhsT=lhs,
                            rhs=qkv_t[:, a + s:b + s],
                            start=(j == 0), stop=(j == len(ops) - 1),
                        )
                    yt = t8 * 8
                    # merge tap (1,1): dw = w*qkv[y+1,x+1] + psum (drains)
                    ya, yb = yt, min(yt + 8, 63)
                    nc.vector.scalar_tensor_tensor(
                        out=dw3[:, ya:yb, 0:63],
                        in0=qk3[:, ya + 1:yb + 1, 1:64],
                        scalar=wm,
                        in1=pd3[:, 0:yb - yt, 0:63],
                        op0=OP.mult, op1=OP.add,
                    )
                    # x=63 col: drain PSUM minus tap(0,1) row-wrap
                    nc.vector.scalar_tensor_tensor(
                        out=dw3[:, yt:yb, 63:64],
                        in0=qk3[:, yt + 1:yb + 1, 0:1],
                        scalar=w01, in1=pd3[:, 0:yb - yt, 63:64],
                        op0=OP.mult, op1=OP.add,
                    )
                    if t8 == 7:
                        nc.scalar.copy(out=dw3[:, 63:64, :],
                                       in_=pd3[:, 7:8, :])
                # x=0 col: subtract tap(0,-1) row-wrap (whole block, in place)
                w0m = wdw_s[:, 54 + ob * 9 + 3:54 + ob * 9 + 4]
                nc.vector.scalar_tensor_tensor(
                    out=dw3[:, 1:64, 0:1], in0=qk3[:, 0:63, 63:64],
                    scalar=w0m, in1=dw3[:, 1:64, 0:1],
                    op0=OP.mult, op1=OP.add,
                )

                if pe_corner:
                    # wrap fixups for the PE corner taps (subtract the
                    # spurious x-wrap column contributions)
                    # (-1,-1) s=-65: out(y,0) wrongly read (y-2,63)
                    wn = wdw_s[:, 54 + ob * 9 + 0:54 + ob * 9 + 1]
                    nc.vector.scalar_tensor_tensor(
                        out=dw3[:, 2:64, 0:1], in0=qk3[:, 0:62, 63:64],
                        scalar=wn, in1=dw3[:, 2:64, 0:1],
                        op0=OP.mult, op1=OP.add)
                    # (1,-1) s=+63: out(y,0) wrongly read (y,63)
                    wn6 = wdw_s[:, 54 + ob * 9 + 6:54 + ob * 9 + 7]
                    nc.vector.scalar_tensor_tensor(
                        out=dw3[:, 0:64, 0:1], in0=qk3[:, 0:64, 63:64],
                        scalar=wn6, in1=dw3[:, 0:64, 0:1],
                        op0=OP.mult, op1=OP.add)
                # remaining corner taps: tensor_scalar (4x) + tensor_tensor
                for (dy, dx) in ([(-1, 1)] if pe_corner else DVE_TAPS):
                    ti = (dy + 1) * 3 + (dx + 1)
                    w_ap = wdw_s[:, ob * 9 + ti:ob * 9 + ti + 1]
                    y0, y1 = max(0, -dy), 64 - max(0, dy)
                    x0, x1 = max(0, -dx), 64 - max(0, dx)
                    sc_t = scr_p.tile([128, N], bf16, tag="sqscr",
                                      name=f"scr{ob}_{ti}")
                    sc3 = sc_t.rearrange("p (y x) -> p y x", y=H)
                    nc.vector.tensor_scalar_mul(
                        sc3[:, y0:y1, x0:x1],
                        qk3[:, y0 + dy:y1 + dy, x0 + dx:x1 + dx], w_ap)
                    nc.vector.tensor_tensor(
                        out=dw3[:, y0:y1, x0:x1], in0=dw3[:, y0:y1, x0:x1],
                        in1=sc3[:, y0:y1, x0:x1], op=OP.add)

                # l2 norm: ssq -> rn = exp(-0.5*ln(ssq) [+ ln(T)]), all in
                # the natural_log_exp ACT table set
                sq = scr_p.tile([128, N], bf16, tag="sqscr")
                ssq = small_p.tile([128, 1], f32, tag="ssq")
                nc.scalar.activation(out=sq, in_=dw_t, func=AF.Square,
                                     accum_out=ssq)
                lg = small_p.tile([128, 1], f32, tag="lg")
                nc.scalar.activation(out=lg, in_=ssq, func=AF.Ln)
                if ob < 2:
                    # q: T/|q| applied later as the softmax-exp scale — the
                    # transposes below don't wait on the norm chain
                    rn = small_p.tile([128, 1], f32, tag=f"rnq{ob}")
                    nc.scalar.activation(out=rn, in_=lg, func=AF.Exp,
                                         scale=-0.5,
                                         bias=lntv_s[:, ob:ob + 1])
                    rnq_v[ob] = rn
                    rhs_ops[ob] = ident
                else:
                    # k: scale must be in kT before the gram — fold into the
                    # transpose matmul via D = diag(rn)
                    rn = small_p.tile([128, 1], f32, tag="rn")
                    nc.scalar.activation(out=rn, in_=lg, func=AF.Exp,
                                         scale=-0.5)
                    D_t = dg_p.tile([128, 128], bf16, tag="D")
                    nc.vector.tensor_scalar_mul(D_t, ident, rn)
                    rhs_ops[ob] = D_t

            def tp_phase(ob):
                dw_t = dw_tiles[ob]
                rhs_op = rhs_ops[ob]
                dst = qT_s if ob < 2 else kT_s
                cof = (ob % 2) * 128
                for g in range(8):
                    tp_t = psml.tile([128, 512], f32, tag="tp")
                    for i in range(4):
                        nb = g * 4 + i
                        # regular matmul: out = dw_chunk.T @ rhs_op — a
                        # transpose that (for k) applies the row scale
                        # (transpose-mode ignores the operand values)
                        nc.tensor.matmul(
                            tp_t[:, i * 128:(i + 1) * 128],
                            lhsT=dw_t[:, nb * 128:(nb + 1) * 128],
                            rhs=rhs_op, start=True, stop=True)
                    nc.scalar.copy(
                        out=dst[:, g * 4:g * 4 + 4, cof:cof + 128],
                        in_=tp_t.rearrange("p (a b) -> p a b", a=4))

            def do_gram(g):
                # raw gram (q unnormalized; k pre-scaled); softmax as single
                # full-row ops with T*rn_q folded into the exp scale and
                # cross-head entries killed by a block-diagonal mask
                pg = psml.tile([128, 128], f32, tag="tp")
                co = g * 128
                for nb in range(32):
                    nc.tensor.matmul(
                        pg,
                        lhsT=qT_s[:, nb, co:co + 128],
                        rhs=kT_s[:, nb, co:co + 128],
                        start=(nb == 0), stop=(nb == 31),
                    )
                rnq = rnq_v[g]
                mx = small_p.tile([128, 1], f32, tag="mx")
                nc.vector.tensor_reduce(out=mx, in_=pg,
                                        axis=mybir.AxisListType.X, op=OP.max)
                ngm = small_p.tile([128, 1], f32, tag="ngm")
                nc.vector.tensor_scalar(out=ngm, in0=mx, scalar1=rnq,
                                        scalar2=-1.0, op0=OP.mult,
                                        op1=OP.mult)
                A_t = a_p.tile([128, 128], bf16, tag="A")
                nc.scalar.activation(out=A_t, in_=pg, func=AF.Exp,
                                     scale=rnq, bias=ngm)
                nc.vector.tensor_tensor(out=A_t, in0=A_t, in1=hmask_s,
                                        op=OP.mult)
                sm = small_p.tile([128, 1], f32, tag="sm")
                nc.vector.tensor_reduce(out=sm, in_=A_t,
                                        axis=mybir.AxisListType.X, op=OP.add)
                rs = small_p.tile([128, 1], f32, tag="rs")
                nc.vector.reciprocal(rs, sm)
                rs_v[g] = rs
                pa = psml.tile([128, 128], bf16, tag="tp")
                nc.tensor.transpose(pa, A_t, ident)
                At = a_p.tile([128, 128], bf16, tag="At")
                nc.scalar.copy(out=At, in_=pa)
                At_v[g] = At

            ALL_TAPS = [(0, 0), (-1, -1), (-1, 0), (-1, 1), (0, -1),
                        (0, 1), (1, -1), (1, 0), (1, 1)]

            def do_av(g):
                # dwconv folded into attention: out = sum_t shift_t(B_t @ v)
                # with B_t[d,c] = At[d,c] * w_dw[v-chan d, tap t]
                Bts = []
                for t9, (dy, dx) in enumerate(ALL_TAPS):
                    Bt = b_p.tile([128, 128], bf16, tag="Bt",
                                  name=f"B{g}_{t9}")
                    wcol = (4 + g) * 9 + (dy + 1) * 3 + (dx + 1)
                    nc.vector.tensor_scalar_mul(
                        Bt, At_v[g], wdw_s[:, wcol:wcol + 1])
                    Bts.append(Bt)
                v3 = qkv_v[g].rearrange("p (y x) -> p y x", y=H)
                for t8 in range(8):
                    yt = t8 * 8
                    pv = pdw.tile([128, TS], f32, tag="pdw", name="pv")
                    pv3 = pv.rearrange("p (y x) -> p y x", y=8)
                    ops = []
                    for t9, (dy, dx) in enumerate(ALL_TAPS):
                        ya, yb = max(yt, -dy), min(yt + 8, 64 - dy)
                        xa, xb = max(0, -dx), 64 - max(0, dx)
                        if ya < yb:
                            ops.append((t9, dy, dx, ya, yb, xa, xb))
                    for j, (t9, dy, dx, ya, yb, xa, xb) in enumerate(ops):
                        nc.tensor.matmul(
                            pv3[:, ya - yt:yb - yt, xa:xb],
                            lhsT=Bts[t9],
                            rhs=v3[:, ya + dy:yb + dy, xa + dx:xb + dx],
                            start=(j == 0), stop=(j == len(ops) - 1))
                    nc.scalar.mul(attn8[:, g, yt * 64:(yt + 8) * 64],
                                  pv, rs_v[g])

            # software-pipelined emission: each engine's stream executes in
            # order, so later-phase PE work (transposes/gram/av) is emitted
            # only once enough independent PE work precedes it to cover the
            # DVE/ACT chains it waits on
            qkv_phase(2)
            tap_phase(2)
            qkv_phase(0)
            tap_phase(0)
            qkv_phase(4)
            qkv_phase(3)
            tap_phase(3)
            tp_phase(2)
            tp_phase(0)
            do_gram(0)
            qkv_phase(1)
            tap_phase(1)
            qkv_phase(5)
            tp_phase(3)
            do_av(0)
            tp_phase(1)
            do_gram(1)
            do_av(1)

            # ---- streamed tail ----
            # residuals are folded into PSUM via identity matmuls, so the
            # per-tile chain is PE -> ACT -> PE -> ACT (no DVE hops)
            x1b = [dwv_p.tile([128, N], bf16, tag="dwv", name=f"x1b{i}")
                   for i in range(2)]
            ys_t = ysp.tile([128, 3, N], f8, tag="ysf8", name="ys")
            # ones-row at hidden index 307 (kb2-local row 51): the mlp2
            # weight row there holds 16*b2, folding the bias into the matmul
            nc.vector.memset(ys_t[:, 2, :], 1.0)

            def proj_phase(t):
                sl = slice(t * 1024, (t + 1) * 1024)
                for ob in range(2):
                    pp = pbig.tile([128, 1024], f32, tag="pbig", name="pp")
                    for h in range(2):
                        nc.tensor.matmul(
                            pp[:, h * TS:(h + 1) * TS],
                            lhsT=wproj_s[:, :, ob * 128:(ob + 1) * 128],
                            rhs=attn8[:, :, t * 1024 + h * TS:
                                      t * 1024 + (h + 1) * TS],
                            perf_mode=DR, start=True, stop=True)
                    nc.vector.scalar_tensor_tensor(
                        out=x1b[ob][:, sl], in0=pp, scalar=1.0 / 16.0,
                        in1=xr[:, ob, sl], op0=OP.mult, op1=OP.add)

            def mlp1_phase(t):
                sl = slice(t * 1024, (t + 1) * 1024)
                for mb in range(3):
                    rows = 128 if mb < 2 else HID - 256
                    pm = pbig.tile([128, 1024], f32, tag="pbig", name="pm")
                    for h in range(2):
                        for kb in range(2):
                            nc.tensor.matmul(
                                pm[:rows, h * TS:(h + 1) * TS],
                                lhsT=wm1_s[:, kb, mb * 128:mb * 128 + rows],
                                rhs=x1b[kb][:, t * 1024 + h * TS:
                                            t * 1024 + (h + 1) * TS],
                                start=(kb == 0), stop=(kb == 1))
                    nc.scalar.activation(
                        out=ys_t[:rows, mb, sl],
                        in_=pm[:rows, :], func=AF.Gelu_apprx_tanh,
                        bias=b1_s[:rows, mb:mb + 1])

            def mlp2_phase(t):
                sl = slice(t * 1024, (t + 1) * 1024)
                for ob in range(2):
                    pm2 = pbig.tile([128, 1024], f32, tag="pbig", name="pm2")
                    for h in range(2):
                        hs = slice(t * 1024 + h * TS, t * 1024 + (h + 1) * TS)
                        nc.tensor.matmul(
                            pm2[:, h * TS:(h + 1) * TS],
                            lhsT=wm2_s[:, 0:2, ob * 128:(ob + 1) * 128],
                            rhs=ys_t[:, 0:2, hs],
                            perf_mode=DR, start=True, stop=False)
                        nc.tensor.matmul(
                            pm2[:, h * TS:(h + 1) * TS],
                            lhsT=wm2_s[:52, 2, ob * 128:(ob + 1) * 128],
                            rhs=ys_t[:52, 2, hs],
                            start=False, stop=True)
                    ot = out_p.tile([128, 1024], f32, tag="ot",
                                    name=f"ot{t}_{ob}")
                    nc.vector.scalar_tensor_tensor(
                        out=ot, in0=pm2,
                        scalar=1.0 / 16.0, in1=x1b[ob][:, sl],
                        op0=OP.mult, op1=OP.add)
                    nc.sync.dma_start(out=out_d[:, ob, sl], in_=ot)

            # pipelined emission: every PE group's ACT dependency is covered
            # by the preceding PE group
            proj_phase(0)
            proj_phase(1)
            mlp1_phase(0)
            mlp1_phase(1)
            mlp2_phase(0)
            proj_phase(2)
            mlp1_phase(2)
            mlp2_phase(1)
            proj_phase(3)
            mlp1_phase(3)
            mlp2_phase(2)
            mlp2_phase(3)

    return nc


def _prep_shared(w_qkv, w_dw, temperature, w_proj, w_mlp1, b_mlp1, w_mlp2, b_mlp2):
    f32 = np.float32
    shared = {}
    F8 = ml_dtypes.float8_e4m3
    shared["wqkv8"] = np.ascontiguousarray(
        (w_qkv.T * 16.0).reshape(2, 128, 3 * C).transpose(1, 0, 2)).astype(F8)
    wd = np.zeros((128, len(PE_TAPS), 4, 128), BF16)
    for ti, (dy, dx) in enumerate(PE_TAPS):
        for cb in range(4):
            w = w_dw[cb * 128:(cb + 1) * 128, 0, dy + 1, dx + 1].astype(f32)
            wd[:, ti, cb, :] = np.diag(w).astype(BF16)
    shared["wdiag"] = wd
    wd2 = np.zeros((128, 2, 2, 128), BF16)
    for tj, (dy, dx) in enumerate(PE2_TAPS):
        for bi, cb in enumerate((1, 3)):
            w = w_dw[cb * 128:(cb + 1) * 128, 0, dy + 1, dx + 1].astype(f32)
            wd2[:, tj, bi, :] = np.diag(w).astype(BF16)
    shared["wdiag2"] = wd2
    hm = np.zeros((128, 128), f32)
    for h4 in range(4):
        hm[h4 * 32:(h4 + 1) * 32, h4 * 32:(h4 + 1) * 32] = 1.0
    shared["hmask"] = hm.astype(BF16)
    wt = np.zeros((128, NB_QKV * 9 * 2), f32)
    for cb in range(NB_QKV):
        for t in range(9):
            wt[:, cb * 9 + t] = w_dw[cb * 128:(cb + 1) * 128, 0, t // 3, t % 3]
    wt[:, 54:] = -wt[:, :54]
    shared["wdw"] = wt
    shared["wproj8"] = np.ascontiguousarray(
        (w_proj.T * 16.0).reshape(2, 128, C).transpose(1, 0, 2)).astype(F8)
    shared["wm1T"] = np.ascontiguousarray(
        w_mlp1.T.reshape(2, 128, HID).transpose(1, 0, 2)).astype(BF16)
    w2 = np.zeros((384, C), f32)
    w2[:HID] = w_mlp2.T * 16.0
    w2[307] = b_mlp2 * 16.0     # ones-row in ys folds the bias in
    shared["wm28"] = np.ascontiguousarray(
        w2.reshape(3, 128, C).transpose(1, 0, 2)).astype(F8)
    b1 = np.zeros((384,), f32)
    b1[:HID] = b_mlp1
    shared["b1"] = np.ascontiguousarray(b1.reshape(3, 128).T)
    t = temperature.reshape(NH).astype(f32)
    tv = np.zeros((128, 2), f32)
    for g in range(2):
        tv[:, g] = np.repeat(t[g * 4:(g + 1) * 4], 32)
    shared["lntv"] = np.log(np.maximum(tv, 1e-30)).astype(f32)
    return shared


def kernel(x, w_qkv, w_dw, temperature, w_proj, w_mlp1, b_mlp1, w_mlp2, b_mlp2,
           _trace=False):
    from concourse.bass_utils import run_bass_kernel_spmd

    if "nc" not in _CACHE:
        nc = _build_bass()
        nc.finalize()
        _CACHE["nc"] = nc
    nc = _CACHE["nc"]

    x = np.asarray(x, np.float32)
    B = x.shape[0]
    shared = _prep_shared(
        np.asarray(w_qkv, np.float32), np.asarray(w_dw, np.float32),
        np.asarray(temperature, np.float32), np.asarray(w_proj, np.float32),
        np.asarray(w_mlp1, np.float32), np.asarray(b_mlp1, np.float32),
        np.asarray(w_mlp2, np.float32), np.asarray(b_mlp2, np.float32))

    in_maps = []
    for i in range(B):
        m = dict(shared)
        xi = np.ascontiguousarray(x[i].reshape(2, 128, N).transpose(1, 0, 2))
        m["xb"] = xi.astype(BF16)
        m["xf8"] = xi.astype(ml_dtypes.float8_e4m3)
        in_maps.append(m)

    res = run_bass_kernel_spmd(nc, in_maps, core_ids=list(range(B)),
                               trace=_trace)
    outs = np.stack([
        r["out"].transpose(1, 0, 2).reshape(C, H, W) for r in res.results
    ])
    if _trace:
        _CACHE["last_exec_ns"] = res.exec_time_ns
        _CACHE["last_profile"] = res.profile_json
    return outs



# revision 68
# speedup vs baseline: 1.0704x; 1.0437x over previous
"""Trainium2 Bass kernel for nn_CustomABlock (MDTA transformer block).

Per-core layout: one batch image [C=256, N=4096(=64x64)] per NeuronCore,
data-parallel over B=8 across 8 cores, all params replicated.

Engine plan (per core):
  PE   : qkv matmul (f32r), 2 dwconv taps (diag matmul), q/k transposes,
         gram (attn logits), attn@v, proj, mlp1, mlp2
  DVE  : 6 dwconv taps (scalar_tensor_tensor FMA, bf16), residual adds,
         reciprocals, row-max reduces, x1 bf16 copy
  ACT  : PSUM drains, l2norm squares (accum), exp (softmax), gelu+bias
  GPSIMD: 1 dwconv tap, identity build
"""

import numpy as np
import ml_dtypes

BF16 = ml_dtypes.bfloat16

C = 256          # dim
N = 4096         # 64*64
H = W = 64
NH = 8           # heads
CH = 32          # channels per head
HID = 307        # mlp hidden
NB_QKV = 6       # qkv channel blocks of 128
NT = 8           # n tiles of 512
TS = 512

# tap index t = (dy+1)*3 + (dx+1)
PE_TAPS = [(0, 0), (-1, 0), (1, 0), (0, -1), (0, 1)]  # PE diag matmuls into PSUM
MERGE_TAP = (1, 1)                  # DVE STT: tap + PSUM drain in one op
DVE_TAPS = [(-1, -1), (-1, 1), (1, -1)]   # DVE tensor_scalar + tensor_tensor
PE2_TAPS = [(-1, -1), (1, -1)]      # extra PE corner taps for late blocks 1/3

_CACHE = {}


def _build_bass():
    import concourse.bass as bass
    from concourse import bacc
    from concourse import mybir
    from concourse.tile import TileContext
    from concourse.masks import make_identity

    # Steer the act-table-load pass: hide Exp/Ln from every set except
    # natural_log_exp_and_others so all transcendentals (norm ln/exp +
    # softmax exp) share one table load instead of ping-ponging sets.
    # Set order (= act_func_set_id) is preserved; the chosen set really
    # does contain both functions, so the loads stay correct.
    if not getattr(bacc, "_act_tables_patched", False):
        _orig_tables = bacc.get_activation_tables
        AF_ = mybir.ActivationFunctionType

        def _patched_tables(arch):
            tabs = {k: set(v) for k, v in _orig_tables(arch).items()}
            for name, fns in tabs.items():
                if name != "natural_log_exp_and_others":
                    fns.discard(AF_.Exp)
                    fns.discard(AF_.Ln)
            return tabs

        bacc.get_activation_tables = _patched_tables
        bacc._act_tables_patched = True

    dt = mybir.dt
    f32 = dt.float32
    f32r = dt.float32r
    bf16 = dt.bfloat16
    AF = mybir.ActivationFunctionType
    OP = mybir.AluOpType

    nc = bacc.Bacc("TRN2")

    f8 = dt.float8e4

    # ---- DRAM I/O (per-core) ----
    xb_d = nc.dram_tensor("xb", [128, 2, N], bf16, kind="ExternalInput")
    xf8_d = nc.dram_tensor("xf8", [128, 2, N], f8, kind="ExternalInput")
    wqkv8_d = nc.dram_tensor("wqkv8", [128, 2, 3 * C], f8, kind="ExternalInput")
    wdiag_d = nc.dram_tensor("wdiag", [128, len(PE_TAPS), 4, 128], bf16,
                             kind="ExternalInput")
    wdiag2_d = nc.dram_tensor("wdiag2", [128, 2, 2, 128], bf16,
                              kind="ExternalInput")
    hmask_d = nc.dram_tensor("hmask", [128, 128], bf16, kind="ExternalInput")
    wdw_d = nc.dram_tensor("wdw", [128, NB_QKV * 9 * 2], f32, kind="ExternalInput")
    wproj_d = nc.dram_tensor("wproj8", [128, 2, C], f8, kind="ExternalInput")
    wm1_d = nc.dram_tensor("wm1T", [128, 2, HID], bf16, kind="ExternalInput")
    wm2_d = nc.dram_tensor("wm28", [128, 3, C], f8, kind="ExternalInput")
    b1_d = nc.dram_tensor("b1", [128, 3], f32, kind="ExternalInput")
    lntv_d = nc.dram_tensor("lntv", [128, 2], f32, kind="ExternalInput")
    out_d = nc.dram_tensor("out", [128, 2, N], f32, kind="ExternalOutput")

    with TileContext(nc) as tc:
        with (
            tc.tile_pool(name="wpool", bufs=1) as wpool,
            tc.tile_pool(name="xpool", bufs=1) as xpool,
            tc.tile_pool(name="qkvp", bufs=4) as qkvp,       # qkv_s blocks / ys reuse
            tc.tile_pool(name="dwqk", bufs=4) as dwqk_p,     # dw q/k blocks
            tc.tile_pool(name="dwv", bufs=2) as dwv_p,       # x1b tiles
            tc.tile_pool(name="qt", bufs=1) as qt_p,
            tc.tile_pool(name="scr", bufs=2) as scr_p,
            tc.tile_pool(name="small", bufs=10) as small_p,
            tc.tile_pool(name="dg", bufs=2) as dg_p,
            tc.tile_pool(name="bt", bufs=18) as b_p,
            tc.tile_pool(name="attn", bufs=1) as atn_p,
            tc.tile_pool(name="ysp", bufs=1) as ysp,
            tc.tile_pool(name="outp", bufs=2) as out_p,
            tc.tile_pool(name="apool", bufs=2) as a_p,
            tc.tile_pool(name="pbig", bufs=2, space="PSUM") as pbig,
            tc.tile_pool(name="pdw", bufs=2, space="PSUM") as pdw,
            tc.tile_pool(name="psml", bufs=2, space="PSUM") as psml,
        ):
            # dummy Ln on an always-ready memset tile: makes the FIRST
            # act-table load the natural_log_exp set (which also covers
            # copy/square); emitted before any other ACT-stream work
            zz = small_p.tile([128, 1], f32, tag="zz")
            nc.vector.memset(zz, 1.0)
            dmy = small_p.tile([128, 1], f32, tag="dmy")
            nc.scalar.activation(out=dmy, in_=zz, func=AF.Ln)

            # ---- load weights & x (critical path first) ----
            xr = xpool.tile([128, 2, N], bf16)
            xf8 = xpool.tile([128, 2, N], f8)
            wqkv_s = wpool.tile([128, 2, 3 * C], f8)
            for kb in range(2):
                nc.sync.dma_start(out=wqkv_s[:, kb, :], in_=wqkv8_d[:, kb, :])
            # x fp8 chunks t-ordered so the first qkv matmuls start early;
            # split across the two HWDGE rings (SP + ACT) to halve the
            # serial startup latency
            for t in range(4):
                for kb in range(2):
                    nc.sync.dma_start(
                        out=xf8[:, kb, t * 1024:(t + 1) * 1024],
                        in_=xf8_d[:, kb, t * 1024:(t + 1) * 1024])
            wdiag_s = wpool.tile([128, len(PE_TAPS), 4, 128], bf16)
            nc.scalar.dma_start(out=wdiag_s, in_=wdiag_d[:, :, :, :])
            wdiag2_s = wpool.tile([128, 2, 2, 128], bf16)
            nc.scalar.dma_start(out=wdiag2_s, in_=wdiag2_d[:, :, :, :])
            wdw_s = wpool.tile([128, NB_QKV * 9 * 2], f32)
            nc.scalar.dma_start(out=wdw_s, in_=wdw_d[:, :])
            # xr (bf16 residual base) is only needed by the tail
            for kb in range(2):
                nc.sync.dma_start(out=xr[:, kb, :], in_=xb_d[:, kb, :])
            # tail-phase weights go via the idle GPSIMD's SWDGE ring so the
            # ACT stream isn't blocked by DMA-issue instructions
            hmask_s = wpool.tile([128, 128], bf16)
            nc.gpsimd.dma_start(out=hmask_s[:, :], in_=hmask_d[:, :])
            wproj_s = wpool.tile([128, 2, C], f8)
            nc.gpsimd.dma_start(out=wproj_s, in_=wproj_d[:, :, :])
            wm1_s = wpool.tile([128, 2, HID], bf16)
            nc.gpsimd.dma_start(out=wm1_s, in_=wm1_d[:, :, :])
            wm2_s = wpool.tile([128, 3, C], f8)
            nc.gpsimd.dma_start(out=wm2_s, in_=wm2_d[:, :, :])
            b1_s = wpool.tile([128, 3], f32)
            nc.gpsimd.dma_start(out=b1_s, in_=b1_d[:, :])
            lntv_s = wpool.tile([128, 2], f32)
            nc.scalar.dma_start(out=lntv_s, in_=lntv_d[:, :])

            ident = wpool.tile([128, 128], bf16)
            make_identity(nc, ident)

            # ---- per-block pipeline ----
            dw_tiles = [None] * NB_QKV
            qT_s = qt_p.tile([128, 32, C], bf16, tag="qT")
            kT_s = qt_p.tile([128, 32, C], bf16, tag="kT")
            attn8 = atn_p.tile([128, 2, N], f8, tag="attn")
            rs_v = [None, None]
            At_v = [None, None]
            rnq_v = [None, None]

            qkv_v = [None, None]   # v_lin tiles for the av-fold
            qkv_tiles = {}
            rhs_ops = {}

            DR = mybir.MatmulPerfMode.DoubleRow

            def qkv_phase(ob):
                # qkv = W_qkv @ x: fp8 DoubleRow folds the K=256 contraction
                # into one matmul (weights pre-scaled x16; drain undoes it)
                qkv_t = qkvp.tile([128, N], bf16, tag="qkv", name=f"qkv{ob}")
                for t in range(4):
                    ps = pbig.tile([128, 1024], f32, tag="pbig", name="ps")
                    for h in range(2):
                        nc.tensor.matmul(
                            ps[:, h * TS:(h + 1) * TS],
                            lhsT=wqkv_s[:, :, ob * 128:(ob + 1) * 128],
                            rhs=xf8[:, :, t * 1024 + h * TS:
                                    t * 1024 + (h + 1) * TS],
                            perf_mode=DR, start=True, stop=True,
                        )
                    nc.scalar.mul(qkv_t[:, t * 1024:(t + 1) * 1024], ps,
                                  1.0 / 16.0)
                qkv_tiles[ob] = qkv_t
                if ob >= 4:
                    qkv_v[ob - 4] = qkv_t

            def tap_phase(ob):
                # dwconv: PE diag taps (flat shifts) -> PSUM[128,512],
                # merge/corner taps + wrap fixups on DVE, then the l2 norm
                qkv_t = qkv_tiles[ob]
                dw_t = dwqk_p.tile([128, N], bf16, tag="dwqk", name=f"dw{ob}")
                dw_tiles[ob] = dw_t
                dw3 = dw_t.rearrange("p (y x) -> p y x", y=H)
                qk3 = qkv_t.rearrange("p (y x) -> p y x", y=H)
                dym, dxm = MERGE_TAP
                wm = wdw_s[:, ob * 9 + (dym + 1) * 3 + dxm + 1:
                           ob * 9 + (dym + 1) * 3 + dxm + 2]
                w01 = wdw_s[:, 54 + ob * 9 + 5:54 + ob * 9 + 6]
                # blocks 1/3 run late: move 2 corner taps to PE to shorten
                # their serial DVE chain (wrap-fixup columns handled below)
                pe_corner = ob in (1, 3)
                for t8 in range(8):
                    pd = pdw.tile([128, TS], f32, tag="pdw", name="pd")
                    pd3 = pd.rearrange("p (y x) -> p y x", y=8)
                    c0 = t8 * TS
                    ops = []
                    for ti, (dy, dx) in enumerate(PE_TAPS):
                        s = dy * 64 + dx
                        a = max(c0, -s)
                        b = min(c0 + TS, N - max(0, s))
                        if a < b:
                            ops.append(((0, ti), s, a, b))
                    if pe_corner:
                        for tj, (dy, dx) in enumerate(PE2_TAPS):
                            s = dy * 64 + dx
                            a = max(c0, -s)
                            b = min(c0 + TS, N - max(0, s))
                            if a < b:
                                ops.append(((1, tj), s, a, b))
                    for j, ((bank, ti), s, a, b) in enumerate(ops):
                        lhs = (wdiag_s[:, ti, ob, :] if bank == 0 else
                               wdiag2_s[:, ti, (ob - 1) // 2, :])
                        nc.tensor.matmul(
                            pd[:, a - c0:b - c0],
                            lhsT=lhs,
                            rhs=qkv_t[:, a + s:b + s],
                            start=(j == 0), stop=(j == len(ops) - 1),
                        )
                    yt = t8 * 8
                    # merge tap (1,1): dw = w*qkv[y+1,x+1] + psum (drains)
                    ya, yb = yt, min(yt + 8, 63)
                    nc.vector.scalar_tensor_tensor(
                        out=dw3[:, ya:yb, 0:63],
                        in0=qk3[:, ya + 1:yb + 1, 1:64],
                        scalar=wm,
                        in1=pd3[:, 0:yb - yt, 0:63],
                        op0=OP.mult, op1=OP.add,
                    )
                    # x=63 col: drain PSUM minus tap(0,1) row-wrap
                    nc.vector.scalar_tensor_tensor(
                        out=dw3[:, yt:yb, 63:64],
                        in0=qk3[:, yt + 1:yb + 1, 0:1],
                        scalar=w01, in1=pd3[:, 0:yb - yt, 63:64],
                        op0=OP.mult, op1=OP.add,
                    )
                    if t8 == 7:
                        nc.scalar.copy(out=dw3[:, 63:64, :],
                                       in_=pd3[:, 7:8, :])
                # x=0 col: subtract tap(0,-1) row-wrap (whole block, in place)
                w0m = wdw_s[:, 54 + ob * 9 + 3:54 + ob * 9 + 4]
                nc.vector.scalar_tensor_tensor(
                    out=dw3[:, 1:64, 0:1], in0=qk3[:, 0:63, 63:64],
                    scalar=w0m, in1=dw3[:, 1:64, 0:1],
                    op0=OP.mult, op1=OP.add,
                )

                if pe_corner:
                    # wrap fixups for the PE corner taps (subtract the
                    # spurious x-wrap column contributions)
                    # (-1,-1) s=-65: out(y,0) wrongly read (y-2,63)
                    wn = wdw_s[:, 54 + ob * 9 + 0:54 + ob * 9 + 1]
                    nc.vector.scalar_tensor_tensor(
                        out=dw3[:, 2:64, 0:1], in0=qk3[:, 0:62, 63:64],
                        scalar=wn, in1=dw3[:, 2:64, 0:1],
                        op0=OP.mult, op1=OP.add)
                    # (1,-1) s=+63: out(y,0) wrongly read (y,63)
                    wn6 = wdw_s[:, 54 + ob * 9 + 6:54 + ob * 9 + 7]
                    nc.vector.scalar_tensor_tensor(
                        out=dw3[:, 0:64, 0:1], in0=qk3[:, 0:64, 63:64],
                        scalar=wn6, in1=dw3[:, 0:64, 0:1],
                        op0=OP.mult, op1=OP.add)
                # remaining corner taps: tensor_scalar (4x) + tensor_tensor
                for (dy, dx) in ([(-1, 1)] if pe_corner else DVE_TAPS):
                    ti = (dy + 1) * 3 + (dx + 1)
                    w_ap = wdw_s[:, ob * 9 + ti:ob * 9 + ti + 1]
                    y0, y1 = max(0, -dy), 64 - max(0, dy)
                    x0, x1 = max(0, -dx), 64 - max(0, dx)
                    sc_t = scr_p.tile([128, N], bf16, tag="sqscr",
                                      name=f"scr{ob}_{ti}")
                    sc3 = sc_t.rearrange("p (y x) -> p y x", y=H)
                    nc.vector.tensor_scalar_mul(
                        sc3[:, y0:y1, x0:x1],
                        qk3[:, y0 + dy:y1 + dy, x0 + dx:x1 + dx], w_ap)
                    nc.vector.tensor_tensor(
                        out=dw3[:, y0:y1, x0:x1], in0=dw3[:, y0:y1, x0:x1],
                        in1=sc3[:, y0:y1, x0:x1], op=OP.add)

                # l2 norm: ssq -> rn = exp(-0.5*ln(ssq) [+ ln(T)]), all in
                # the natural_log_exp ACT table set
                sq = scr_p.tile([128, N], bf16, tag="sqscr")
                ssq = small_p.tile([128, 1], f32, tag="ssq")
                nc.scalar.activation(out=sq, in_=dw_t, func=AF.Square,
                                     accum_out=ssq)
                lg = small_p.tile([128, 1], f32, tag="lg")
                nc.scalar.activation(out=lg, in_=ssq, func=AF.Ln)
                if ob < 2:
                    # q: T/|q| applied later as the softmax-exp scale — the
                    # transposes below don't wait on the norm chain
                    rn = small_p.tile([128, 1], f32, tag=f"rnq{ob}")
                    nc.scalar.activation(out=rn, in_=lg, func=AF.Exp,
                                         scale=-0.5,
                                         bias=lntv_s[:, ob:ob + 1])
                    rnq_v[ob] = rn
                    rhs_ops[ob] = ident
                else:
                    # k: scale must be in kT before the gram — fold into the
                    # transpose matmul via D = diag(rn)
                    rn = small_p.tile([128, 1], f32, tag="rn")
                    nc.scalar.activation(out=rn, in_=lg, func=AF.Exp,
                                         scale=-0.5)
                    D_t = dg_p.tile([128, 128], bf16, tag="D")
                    nc.vector.tensor_scalar_mul(D_t, ident, rn)
                    rhs_ops[ob] = D_t

            def tp_phase(ob):
                dw_t = dw_tiles[ob]
                rhs_op = rhs_ops[ob]
                dst = qT_s if ob < 2 else kT_s
                cof = (ob % 2) * 128
                for g in range(8):
                    tp_t = psml.tile([128, 512], f32, tag="tp")
                    for i in range(4):
                        nb = g * 4 + i
                        # regular matmul: out = dw_chunk.T @ rhs_op — a
                        # transpose that (for k) applies the row scale
                        # (transpose-mode ignores the operand values)
                        nc.tensor.matmul(
                            tp_t[:, i * 128:(i + 1) * 128],
                            lhsT=dw_t[:, nb * 128:(nb + 1) * 128],
                            rhs=rhs_op, start=True, stop=True)
                    nc.scalar.copy(
                        out=dst[:, g * 4:g * 4 + 4, cof:cof + 128],
                        in_=tp_t.rearrange("p (a b) -> p a b", a=4))

            def do_gram(g):
                # raw gram (q unnormalized; k pre-scaled); softmax as single
                # full-row ops with T*rn_q folded into the exp scale and
                # cross-head entries killed by a block-diagonal mask
                pg = psml.tile([128, 128], f32, tag="tp")
                co = g * 128
                for nb in range(32):
                    nc.tensor.matmul(
                        pg,
                        lhsT=qT_s[:, nb, co:co + 128],
                        rhs=kT_s[:, nb, co:co + 128],
                        start=(nb == 0), stop=(nb == 31),
                    )
                rnq = rnq_v[g]
                mx = small_p.tile([128, 1], f32, tag="mx")
                nc.vector.tensor_reduce(out=mx, in_=pg,
                                        axis=mybir.AxisListType.X, op=OP.max)
                ngm = small_p.tile([128, 1], f32, tag="ngm")
                nc.vector.tensor_scalar(out=ngm, in0=mx, scalar1=rnq,
                                        scalar2=-1.0, op0=OP.mult,
                                        op1=OP.mult)
                A_t = a_p.tile([128, 128], bf16, tag="A")
                nc.scalar.activation(out=A_t, in_=pg, func=AF.Exp,
                                     scale=rnq, bias=ngm)
                nc.vector.tensor_tensor(out=A_t, in0=A_t, in1=hmask_s,
                                        op=OP.mult)
                sm = small_p.tile([128, 1], f32, tag="sm")
                nc.vector.tensor_reduce(out=sm, in_=A_t,
                                        axis=mybir.AxisListType.X, op=OP.add)
                rs = small_p.tile([128, 1], f32, tag="rs")
                nc.vector.reciprocal(rs, sm)
                rs_v[g] = rs
                pa = psml.tile([128, 128], bf16, tag="tp")
                nc.tensor.transpose(pa, A_t, ident)
                At = a_p.tile([128, 128], bf16, tag="At")
                nc.scalar.copy(out=At, in_=pa)
                At_v[g] = At

            ALL_TAPS = [(0, 0), (-1, -1), (-1, 0), (-1, 1), (0, -1),
                        (0, 1), (1, -1), (1, 0), (1, 1)]

            Bts_v = [None, None]

            def do_av_prep(g):
                # dwconv folded into attention: out = sum_t shift_t(B_t @ v)
                # with B_t[d,c] = At[d,c] * w_dw[v-chan d, tap t]
                Bts = []
                for t9, (dy, dx) in enumerate(ALL_TAPS):
                    Bt = b_p.tile([128, 128], bf16, tag="Bt",
                                  name=f"B{g}_{t9}")
                    wcol = (4 + g) * 9 + (dy + 1) * 3 + (dx + 1)
                    nc.vector.tensor_scalar_mul(
                        Bt, At_v[g], wdw_s[:, wcol:wcol + 1])
                    Bts.append(Bt)
                Bts_v[g] = Bts

            def do_av_part(g, t8s):
                Bts = Bts_v[g]
                v3 = qkv_v[g].rearrange("p (y x) -> p y x", y=H)
                for t8 in t8s:
                    yt = t8 * 8
                    pv = pdw.tile([128, TS], f32, tag="pdw", name="pv")
                    pv3 = pv.rearrange("p (y x) -> p y x", y=8)
                    ops = []
                    for t9, (dy, dx) in enumerate(ALL_TAPS):
                        ya, yb = max(yt, -dy), min(yt + 8, 64 - dy)
                        xa, xb = max(0, -dx), 64 - max(0, dx)
                        if ya < yb:
                            ops.append((t9, dy, dx, ya, yb, xa, xb))
                    for j, (t9, dy, dx, ya, yb, xa, xb) in enumerate(ops):
                        nc.tensor.matmul(
                            pv3[:, ya - yt:yb - yt, xa:xb],
                            lhsT=Bts[t9],
                            rhs=v3[:, ya + dy:yb + dy, xa + dx:xb + dx],
                            start=(j == 0), stop=(j == len(ops) - 1))
                    nc.scalar.mul(attn8[:, g, yt * 64:(yt + 8) * 64],
                                  pv, rs_v[g])

            # software-pipelined emission: each engine's stream executes in
            # order, so later-phase PE work (transposes/gram/av) is emitted
            # only once enough independent PE work precedes it to cover the
            # DVE/ACT chains it waits on
            qkv_phase(2)
            tap_phase(2)
            qkv_phase(0)
            tap_phase(0)
            qkv_phase(4)
            qkv_phase(3)
            tap_phase(3)
            tp_phase(2)
            tp_phase(0)
            do_gram(0)
            qkv_phase(1)
            tap_phase(1)
            qkv_phase(5)
            tp_phase(3)
            do_av_prep(0)
            do_av_part(0, range(8))
            tp_phase(1)
            do_gram(1)
            do_av_prep(1)

            # ---- streamed tail ----
            # residuals are folded into PSUM via identity matmuls, so the
            # per-tile chain is PE -> ACT -> PE -> ACT (no DVE hops)
            x1b = [dwv_p.tile([128, N], bf16, tag="dwv", name=f"x1b{i}")
                   for i in range(2)]
            ys_t = ysp.tile([128, 3, N], f8, tag="ysf8", name="ys")
            # ones-row at hidden index 307 (kb2-local row 51): the mlp2
            # weight row there holds 16*b2, folding the bias into the matmul
            nc.vector.memset(ys_t[:, 2, :], 1.0)

            def proj_phase(t):
                sl = slice(t * 1024, (t + 1) * 1024)
                for ob in range(2):
                    pp = pbig.tile([128, 1024], f32, tag="pbig", name="pp")
                    for h in range(2):
                        nc.tensor.matmul(
                            pp[:, h * TS:(h + 1) * TS],
                            lhsT=wproj_s[:, :, ob * 128:(ob + 1) * 128],
                            rhs=attn8[:, :, t * 1024 + h * TS:
                                      t * 1024 + (h + 1) * TS],
                            perf_mode=DR, start=True, stop=True)
                    nc.vector.scalar_tensor_tensor(
                        out=x1b[ob][:, sl], in0=pp, scalar=1.0 / 16.0,
                        in1=xr[:, ob, sl], op0=OP.mult, op1=OP.add)

            def mlp1_phase(t):
                sl = slice(t * 1024, (t + 1) * 1024)
                for mb in range(3):
                    rows = 128 if mb < 2 else HID - 256
                    pm = pbig.tile([128, 1024], f32, tag="pbig", name="pm")
                    for h in range(2):
                        for kb in range(2):
                            nc.tensor.matmul(
                                pm[:rows, h * TS:(h + 1) * TS],
                                lhsT=wm1_s[:, kb, mb * 128:mb * 128 + rows],
                                rhs=x1b[kb][:, t * 1024 + h * TS:
                                            t * 1024 + (h + 1) * TS],
                                start=(kb == 0), stop=(kb == 1))
                    nc.scalar.activation(
                        out=ys_t[:rows, mb, sl],
                        in_=pm[:rows, :], func=AF.Gelu_apprx_tanh,
                        bias=b1_s[:rows, mb:mb + 1])

            def mlp2_phase(t):
                sl = slice(t * 1024, (t + 1) * 1024)
                for ob in range(2):
                    pm2 = pbig.tile([128, 1024], f32, tag="pbig", name="pm2")
                    for h in range(2):
                        hs = slice(t * 1024 + h * TS, t * 1024 + (h + 1) * TS)
                        nc.tensor.matmul(
                            pm2[:, h * TS:(h + 1) * TS],
                            lhsT=wm2_s[:, 0:2, ob * 128:(ob + 1) * 128],
                            rhs=ys_t[:, 0:2, hs],
                            perf_mode=DR, start=True, stop=False)
                        nc.tensor.matmul(
                            pm2[:, h * TS:(h + 1) * TS],
                            lhsT=wm2_s[:52, 2, ob * 128:(ob + 1) * 128],
                            rhs=ys_t[:52, 2, hs],
                            start=False, stop=True)
                    ot = out_p.tile([128, 1024], f32, tag="ot",
                                    name=f"ot{t}_{ob}")
                    nc.vector.scalar_tensor_tensor(
                        out=ot, in0=pm2,
                        scalar=1.0 / 16.0, in1=x1b[ob][:, sl],
                        op0=OP.mult, op1=OP.add)
                    nc.sync.dma_start(out=out_d[:, ob, sl], in_=ot)

            # pipelined emission: av1 chunks woven into the tail's proj
            # stream; every PE group's ACT dependency is covered by the
            # preceding PE group
            do_av_part(1, [0, 1])
            proj_phase(0)
            do_av_part(1, [2, 3])
            proj_phase(1)
            do_av_part(1, [4, 5])
            mlp1_phase(0)
            do_av_part(1, [6, 7])
            proj_phase(2)
            mlp1_phase(1)
            mlp2_phase(0)
            proj_phase(3)
            mlp1_phase(2)
            mlp2_phase(1)
            mlp1_phase(3)
            mlp2_phase(2)
            mlp2_phase(3)

    return nc


def _prep_shared(w_qkv, w_dw, temperature, w_proj, w_mlp1, b_mlp1, w_mlp2, b_mlp2):
    f32 = np.float32
    shared = {}
    F8 = ml_dtypes.float8_e4m3
    shared["wqkv8"] = np.ascontiguousarray(
        (w_qkv.T * 16.0).reshape(2, 128, 3 * C).transpose(1, 0, 2)).astype(F8)
    wd = np.zeros((128, len(PE_TAPS), 4, 128), BF16)
    for ti, (dy, dx) in enumerate(PE_TAPS):
        for cb in range(4):
            w = w_dw[cb * 128:(cb + 1) * 128, 0, dy + 1, dx + 1].astype(f32)
            wd[:, ti, cb, :] = np.diag(w).astype(BF16)
    shared["wdiag"] = wd
    wd2 = np.zeros((128, 2, 2, 128), BF16)
    for tj, (dy, dx) in enumerate(PE2_TAPS):
        for bi, cb in enumerate((1, 3)):
            w = w_dw[cb * 128:(cb + 1) * 128, 0, dy + 1, dx + 1].astype(f32)
            wd2[:, tj, bi, :] = np.diag(w).astype(BF16)
    shared["wdiag2"] = wd2
    hm = np.zeros((128, 128), f32)
    for h4 in range(4):
        hm[h4 * 32:(h4 + 1) * 32, h4 * 32:(h4 + 1) * 32] = 1.0
    shared["hmask"] = hm.astype(BF16)
    wt = np.zeros((128, NB_QKV * 9 * 2), f32)
    for cb in range(NB_QKV):
        for t in range(9):
            wt[:, cb * 9 + t] = w_dw[cb * 128:(cb + 1) * 128, 0, t // 3, t % 3]
    wt[:, 54:] = -wt[:, :54]
    shared["wdw"] = wt
    shared["wproj8"] = np.ascontiguousarray(
        (w_proj.T * 16.0).reshape(2, 128, C).transpose(1, 0, 2)).astype(F8)
    shared["wm1T"] = np.ascontiguousarray(
        w_mlp1.T.reshape(2, 128, HID).transpose(1, 0, 2)).astype(BF16)
    w2 = np.zeros((384, C), f32)
    w2[:HID] = w_mlp2.T * 16.0
    w2[307] = b_mlp2 * 16.0     # ones-row in ys folds the bias in
    shared["wm28"] = np.ascontiguousarray(
        w2.reshape(3, 128, C).transpose(1, 0, 2)).astype(F8)
    b1 = np.zeros((384,), f32)
    b1[:HID] = b_mlp1
    shared["b1"] = np.ascontiguousarray(b1.reshape(3, 128).T)
    t = temperature.reshape(NH).astype(f32)
    tv = np.zeros((128, 2), f32)
    for g in range(2):
        tv[:, g] = np.repeat(t[g * 4:(g + 1) * 4], 32)
    shared["lntv"] = np.log(np.maximum(tv, 1e-30)).astype(f32)
    return shared


def kernel(x, w_qkv, w_dw, temperature, w_proj, w_mlp1, b_mlp1, w_mlp2, b_mlp2,
           _trace=False):
    from concourse.bass_utils import run_bass_kernel_spmd

    if "nc" not in _CACHE:
        nc = _build_bass()
        nc.finalize()
        _CACHE["nc"] = nc
    nc = _CACHE["nc"]

    x = np.asarray(x, np.float32)
    B = x.shape[0]
    shared = _prep_shared(
        np.asarray(w_qkv, np.float32), np.asarray(w_dw, np.float32),
        np.asarray(temperature, np.float32), np.asarray(w_proj, np.float32),
        np.asarray(w_mlp1, np.float32), np.asarray(b_mlp1, np.float32),
        np.asarray(w_mlp2, np.float32), np.asarray(b_mlp2, np.float32))

    in_maps = []
    for i in range(B):
        m = dict(shared)
        xi = np.ascontiguousarray(x[i].reshape(2, 128, N).transpose(1, 0, 2))
        m["xb"] = xi.astype(BF16)
        m["xf8"] = xi.astype(ml_dtypes.float8_e4m3)
        in_maps.append(m)

    res = run_bass_kernel_spmd(nc, in_maps, core_ids=list(range(B)),
                               trace=_trace)
    outs = np.stack([
        r["out"].transpose(1, 0, 2).reshape(C, H, W) for r in res.results
    ])
    if _trace:
        _CACHE["last_exec_ns"] = res.exec_time_ns
        _CACHE["last_profile"] = res.profile_json
    return outs



# revision 69
# speedup vs baseline: 1.0764x; 1.0055x over previous
"""Trainium2 Bass kernel for nn_CustomABlock (MDTA transformer block).

Per-core layout: one batch image [C=256, N=4096(=64x64)] per NeuronCore,
data-parallel over B=8 across 8 cores, all params replicated.

Engine plan (per core):
  PE   : qkv matmul (f32r), 2 dwconv taps (diag matmul), q/k transposes,
         gram (attn logits), attn@v, proj, mlp1, mlp2
  DVE  : 6 dwconv taps (scalar_tensor_tensor FMA, bf16), residual adds,
         reciprocals, row-max reduces, x1 bf16 copy
  ACT  : PSUM drains, l2norm squares (accum), exp (softmax), gelu+bias
  GPSIMD: 1 dwconv tap, identity build
"""

import numpy as np
import ml_dtypes

BF16 = ml_dtypes.bfloat16

C = 256          # dim
N = 4096         # 64*64
H = W = 64
NH = 8           # heads
CH = 32          # channels per head
HID = 307        # mlp hidden
NB_QKV = 6       # qkv channel blocks of 128
NT = 8           # n tiles of 512
TS = 512

# tap index t = (dy+1)*3 + (dx+1)
PE_TAPS = [(0, 0), (-1, 0), (1, 0), (0, -1), (0, 1)]  # PE diag matmuls into PSUM
MERGE_TAP = (1, 1)                  # DVE STT: tap + PSUM drain in one op
DVE_TAPS = [(-1, -1), (-1, 1), (1, -1)]   # DVE tensor_scalar + tensor_tensor
PE2_TAPS = [(-1, -1), (1, -1)]      # extra PE corner taps for late blocks 1/3

_CACHE = {}


def _build_bass():
    import concourse.bass as bass
    from concourse import bacc
    from concourse import mybir
    from concourse.tile import TileContext
    from concourse.masks import make_identity

    # Steer the act-table-load pass: hide Exp/Ln from every set except
    # natural_log_exp_and_others so all transcendentals (norm ln/exp +
    # softmax exp) share one table load instead of ping-ponging sets.
    # Set order (= act_func_set_id) is preserved; the chosen set really
    # does contain both functions, so the loads stay correct.
    if not getattr(bacc, "_act_tables_patched", False):
        _orig_tables = bacc.get_activation_tables
        AF_ = mybir.ActivationFunctionType

        def _patched_tables(arch):
            tabs = {k: set(v) for k, v in _orig_tables(arch).items()}
            for name, fns in tabs.items():
                if name != "natural_log_exp_and_others":
                    fns.discard(AF_.Exp)
                    fns.discard(AF_.Ln)
            return tabs

        bacc.get_activation_tables = _patched_tables
        bacc._act_tables_patched = True

    dt = mybir.dt
    f32 = dt.float32
    f32r = dt.float32r
    bf16 = dt.bfloat16
    AF = mybir.ActivationFunctionType
    OP = mybir.AluOpType

    nc = bacc.Bacc("TRN2")

    f8 = dt.float8e4

    # ---- DRAM I/O (per-core) ----
    xb_d = nc.dram_tensor("xb", [128, 2, N], bf16, kind="ExternalInput")
    xf8_d = nc.dram_tensor("xf8", [128, 2, N], f8, kind="ExternalInput")
    wqkv8_d = nc.dram_tensor("wqkv8", [128, 2, 3 * C], f8, kind="ExternalInput")
    wdiag_d = nc.dram_tensor("wdiag", [128, len(PE_TAPS), 4, 128], bf16,
                             kind="ExternalInput")
    wdiag2_d = nc.dram_tensor("wdiag2", [128, 2, 2, 128], bf16,
                              kind="ExternalInput")
    hmask_d = nc.dram_tensor("hmask", [128, 128], bf16, kind="ExternalInput")
    wdw_d = nc.dram_tensor("wdw", [128, NB_QKV * 9 * 2], f32, kind="ExternalInput")
    wproj_d = nc.dram_tensor("wproj8", [128, 2, C], f8, kind="ExternalInput")
    wm1_d = nc.dram_tensor("wm1T", [128, 2, HID], bf16, kind="ExternalInput")
    wm2_d = nc.dram_tensor("wm28", [128, 3, C], f8, kind="ExternalInput")
    b1_d = nc.dram_tensor("b1", [128, 3], f32, kind="ExternalInput")
    lntv_d = nc.dram_tensor("lntv", [128, 2], f32, kind="ExternalInput")
    out_d = nc.dram_tensor("out", [128, 2, N], f32, kind="ExternalOutput")

    with TileContext(nc) as tc:
        with (
            tc.tile_pool(name="wpool", bufs=1) as wpool,
            tc.tile_pool(name="xpool", bufs=1) as xpool,
            tc.tile_pool(name="qkvp", bufs=4) as qkvp,       # qkv_s blocks / ys reuse
            tc.tile_pool(name="dwqk", bufs=4) as dwqk_p,     # dw q/k blocks
            tc.tile_pool(name="dwv", bufs=2) as dwv_p,       # x1b tiles
            tc.tile_pool(name="qt", bufs=1) as qt_p,
            tc.tile_pool(name="scr", bufs=2) as scr_p,
            tc.tile_pool(name="small", bufs=10) as small_p,
            tc.tile_pool(name="dg", bufs=2) as dg_p,
            tc.tile_pool(name="bt", bufs=18) as b_p,
            tc.tile_pool(name="attn", bufs=1) as atn_p,
            tc.tile_pool(name="ysp", bufs=1) as ysp,
            tc.tile_pool(name="outp", bufs=2) as out_p,
            tc.tile_pool(name="apool", bufs=2) as a_p,
            tc.tile_pool(name="pbig", bufs=2, space="PSUM") as pbig,
            tc.tile_pool(name="pdw", bufs=2, space="PSUM") as pdw,
            tc.tile_pool(name="psml", bufs=2, space="PSUM") as psml,
        ):
            # dummy Ln on an always-ready memset tile: makes the FIRST
            # act-table load the natural_log_exp set (which also covers
            # copy/square); emitted before any other ACT-stream work
            zz = small_p.tile([128, 1], f32, tag="zz")
            nc.vector.memset(zz, 1.0)
            dmy = small_p.tile([128, 1], f32, tag="dmy")
            nc.scalar.activation(out=dmy, in_=zz, func=AF.Ln)

            # ---- load weights & x (critical path first) ----
            xr = xpool.tile([128, 2, N], bf16)
            xf8 = xpool.tile([128, 2, N], f8)
            wqkv_s = wpool.tile([128, 2, 3 * C], f8)
            for kb in range(2):
                nc.sync.dma_start(out=wqkv_s[:, kb, :], in_=wqkv8_d[:, kb, :])
            # x fp8 chunks t-ordered so the first qkv matmuls start early;
            # split across the two HWDGE rings (SP + ACT) to halve the
            # serial startup latency
            for t in range(4):
                # both kb planes in one DMA: the DoubleRow rhs reads both,
                # so this keeps the dependency per-chunk
                nc.sync.dma_start(
                    out=xf8[:, :, t * 1024:(t + 1) * 1024],
                    in_=xf8_d[:, :, t * 1024:(t + 1) * 1024])
            wdiag_s = wpool.tile([128, len(PE_TAPS), 4, 128], bf16)
            nc.scalar.dma_start(out=wdiag_s, in_=wdiag_d[:, :, :, :])
            wdiag2_s = wpool.tile([128, 2, 2, 128], bf16)
            nc.scalar.dma_start(out=wdiag2_s, in_=wdiag2_d[:, :, :, :])
            wdw_s = wpool.tile([128, NB_QKV * 9 * 2], f32)
            nc.scalar.dma_start(out=wdw_s, in_=wdw_d[:, :])
            # xr (bf16 residual base) is only needed by the tail
            for kb in range(2):
                nc.sync.dma_start(out=xr[:, kb, :], in_=xb_d[:, kb, :])
            # tail-phase weights go via the idle GPSIMD's SWDGE ring so the
            # ACT stream isn't blocked by DMA-issue instructions
            hmask_s = wpool.tile([128, 128], bf16)
            nc.gpsimd.dma_start(out=hmask_s[:, :], in_=hmask_d[:, :])
            wproj_s = wpool.tile([128, 2, C], f8)
            nc.gpsimd.dma_start(out=wproj_s, in_=wproj_d[:, :, :])
            wm1_s = wpool.tile([128, 2, HID], bf16)
            nc.gpsimd.dma_start(out=wm1_s, in_=wm1_d[:, :, :])
            wm2_s = wpool.tile([128, 3, C], f8)
            nc.gpsimd.dma_start(out=wm2_s, in_=wm2_d[:, :, :])
            b1_s = wpool.tile([128, 3], f32)
            nc.gpsimd.dma_start(out=b1_s, in_=b1_d[:, :])
            lntv_s = wpool.tile([128, 2], f32)
            nc.scalar.dma_start(out=lntv_s, in_=lntv_d[:, :])

            ident = wpool.tile([128, 128], bf16)
            make_identity(nc, ident)

            # ---- per-block pipeline ----
            dw_tiles = [None] * NB_QKV
            qT_s = qt_p.tile([128, 32, C], bf16, tag="qT")
            kT_s = qt_p.tile([128, 32, C], bf16, tag="kT")
            attn8 = atn_p.tile([128, 2, N], f8, tag="attn")
            rs_v = [None, None]
            At_v = [None, None]
            rnq_v = [None, None]

            qkv_v = [None, None]   # v_lin tiles for the av-fold
            qkv_tiles = {}
            rhs_ops = {}

            DR = mybir.MatmulPerfMode.DoubleRow

            def qkv_phase(ob):
                # qkv = W_qkv @ x: fp8 DoubleRow folds the K=256 contraction
                # into one matmul (weights pre-scaled x16; drain undoes it)
                qkv_t = qkvp.tile([128, N], bf16, tag="qkv", name=f"qkv{ob}")
                for t in range(4):
                    ps = pbig.tile([128, 1024], f32, tag="pbig", name="ps")
                    for h in range(2):
                        nc.tensor.matmul(
                            ps[:, h * TS:(h + 1) * TS],
                            lhsT=wqkv_s[:, :, ob * 128:(ob + 1) * 128],
                            rhs=xf8[:, :, t * 1024 + h * TS:
                                    t * 1024 + (h + 1) * TS],
                            perf_mode=DR, start=True, stop=True,
                        )
                    nc.scalar.mul(qkv_t[:, t * 1024:(t + 1) * 1024], ps,
                                  1.0 / 16.0)
                qkv_tiles[ob] = qkv_t
                if ob >= 4:
                    qkv_v[ob - 4] = qkv_t

            def tap_phase(ob):
                # dwconv: PE diag taps (flat shifts) -> PSUM[128,512],
                # merge/corner taps + wrap fixups on DVE, then the l2 norm
                qkv_t = qkv_tiles[ob]
                dw_t = dwqk_p.tile([128, N], bf16, tag="dwqk", name=f"dw{ob}")
                dw_tiles[ob] = dw_t
                dw3 = dw_t.rearrange("p (y x) -> p y x", y=H)
                qk3 = qkv_t.rearrange("p (y x) -> p y x", y=H)
                dym, dxm = MERGE_TAP
                wm = wdw_s[:, ob * 9 + (dym + 1) * 3 + dxm + 1:
                           ob * 9 + (dym + 1) * 3 + dxm + 2]
                w01 = wdw_s[:, 54 + ob * 9 + 5:54 + ob * 9 + 6]
                # blocks 1/3 run late: move 2 corner taps to PE to shorten
                # their serial DVE chain (wrap-fixup columns handled below)
                pe_corner = ob in (1, 3)
                for t8 in range(8):
                    pd = pdw.tile([128, TS], f32, tag="pdw", name="pd")
                    pd3 = pd.rearrange("p (y x) -> p y x", y=8)
                    c0 = t8 * TS
                    ops = []
                    for ti, (dy, dx) in enumerate(PE_TAPS):
                        s = dy * 64 + dx
                        a = max(c0, -s)
                        b = min(c0 + TS, N - max(0, s))
                        if a < b:
                            ops.append(((0, ti), s, a, b))
                    if pe_corner:
                        for tj, (dy, dx) in enumerate(PE2_TAPS):
                            s = dy * 64 + dx
                            a = max(c0, -s)
                            b = min(c0 + TS, N - max(0, s))
                            if a < b:
                                ops.append(((1, tj), s, a, b))
                    for j, ((bank, ti), s, a, b) in enumerate(ops):
                        lhs = (wdiag_s[:, ti, ob, :] if bank == 0 else
                               wdiag2_s[:, ti, (ob - 1) // 2, :])
                        nc.tensor.matmul(
                            pd[:, a - c0:b - c0],
                            lhsT=lhs,
                            rhs=qkv_t[:, a + s:b + s],
                            start=(j == 0), stop=(j == len(ops) - 1),
                        )
                    yt = t8 * 8
                    # merge tap (1,1): dw = w*qkv[y+1,x+1] + psum (drains)
                    ya, yb = yt, min(yt + 8, 63)
                    nc.vector.scalar_tensor_tensor(
                        out=dw3[:, ya:yb, 0:63],
                        in0=qk3[:, ya + 1:yb + 1, 1:64],
                        scalar=wm,
                        in1=pd3[:, 0:yb - yt, 0:63],
                        op0=OP.mult, op1=OP.add,
                    )
                    # x=63 col: drain PSUM minus tap(0,1) row-wrap
                    nc.vector.scalar_tensor_tensor(
                        out=dw3[:, yt:yb, 63:64],
                        in0=qk3[:, yt + 1:yb + 1, 0:1],
                        scalar=w01, in1=pd3[:, 0:yb - yt, 63:64],
                        op0=OP.mult, op1=OP.add,
                    )
                    if t8 == 7:
                        nc.scalar.copy(out=dw3[:, 63:64, :],
                                       in_=pd3[:, 7:8, :])
                # x=0 col: subtract tap(0,-1) row-wrap (whole block, in place)
                w0m = wdw_s[:, 54 + ob * 9 + 3:54 + ob * 9 + 4]
                nc.vector.scalar_tensor_tensor(
                    out=dw3[:, 1:64, 0:1], in0=qk3[:, 0:63, 63:64],
                    scalar=w0m, in1=dw3[:, 1:64, 0:1],
                    op0=OP.mult, op1=OP.add,
                )

                if pe_corner:
                    # wrap fixups for the PE corner taps (subtract the
                    # spurious x-wrap column contributions)
                    # (-1,-1) s=-65: out(y,0) wrongly read (y-2,63)
                    wn = wdw_s[:, 54 + ob * 9 + 0:54 + ob * 9 + 1]
                    nc.vector.scalar_tensor_tensor(
                        out=dw3[:, 2:64, 0:1], in0=qk3[:, 0:62, 63:64],
                        scalar=wn, in1=dw3[:, 2:64, 0:1],
                        op0=OP.mult, op1=OP.add)
                    # (1,-1) s=+63: out(y,0) wrongly read (y,63)
                    wn6 = wdw_s[:, 54 + ob * 9 + 6:54 + ob * 9 + 7]
                    nc.vector.scalar_tensor_tensor(
                        out=dw3[:, 0:64, 0:1], in0=qk3[:, 0:64, 63:64],
                        scalar=wn6, in1=dw3[:, 0:64, 0:1],
                        op0=OP.mult, op1=OP.add)
                # remaining corner taps: tensor_scalar (4x) + tensor_tensor
                for (dy, dx) in ([(-1, 1)] if pe_corner else DVE_TAPS):
                    ti = (dy + 1) * 3 + (dx + 1)
                    w_ap = wdw_s[:, ob * 9 + ti:ob * 9 + ti + 1]
                    y0, y1 = max(0, -dy), 64 - max(0, dy)
                    x0, x1 = max(0, -dx), 64 - max(0, dx)
                    sc_t = scr_p.tile([128, N], bf16, tag="sqscr",
                                      name=f"scr{ob}_{ti}")
                    sc3 = sc_t.rearrange("p (y x) -> p y x", y=H)
                    nc.vector.tensor_scalar_mul(
                        sc3[:, y0:y1, x0:x1],
                        qk3[:, y0 + dy:y1 + dy, x0 + dx:x1 + dx], w_ap)
                    nc.vector.tensor_tensor(
                        out=dw3[:, y0:y1, x0:x1], in0=dw3[:, y0:y1, x0:x1],
                        in1=sc3[:, y0:y1, x0:x1], op=OP.add)

                # l2 norm: ssq -> rn = exp(-0.5*ln(ssq) [+ ln(T)]), all in
                # the natural_log_exp ACT table set
                sq = scr_p.tile([128, N], bf16, tag="sqscr")
                ssq = small_p.tile([128, 1], f32, tag="ssq")
                nc.scalar.activation(out=sq, in_=dw_t, func=AF.Square,
                                     accum_out=ssq)
                lg = small_p.tile([128, 1], f32, tag="lg")
                nc.scalar.activation(out=lg, in_=ssq, func=AF.Ln)
                if ob < 2:
                    # q: T/|q| applied later as the softmax-exp scale — the
                    # transposes below don't wait on the norm chain
                    rn = small_p.tile([128, 1], f32, tag=f"rnq{ob}")
                    nc.scalar.activation(out=rn, in_=lg, func=AF.Exp,
                                         scale=-0.5,
                                         bias=lntv_s[:, ob:ob + 1])
                    rnq_v[ob] = rn
                    rhs_ops[ob] = ident
                else:
                    # k: scale must be in kT before the gram — fold into the
                    # transpose matmul via D = diag(rn)
                    rn = small_p.tile([128, 1], f32, tag="rn")
                    nc.scalar.activation(out=rn, in_=lg, func=AF.Exp,
                                         scale=-0.5)
                    D_t = dg_p.tile([128, 128], bf16, tag="D")
                    nc.vector.tensor_scalar_mul(D_t, ident, rn)
                    rhs_ops[ob] = D_t

            def tp_phase(ob):
                dw_t = dw_tiles[ob]
                rhs_op = rhs_ops[ob]
                dst = qT_s if ob < 2 else kT_s
                cof = (ob % 2) * 128
                for g in range(8):
                    tp_t = psml.tile([128, 512], f32, tag="tp")
                    for i in range(4):
                        nb = g * 4 + i
                        # regular matmul: out = dw_chunk.T @ rhs_op — a
                        # transpose that (for k) applies the row scale
                        # (transpose-mode ignores the operand values)
                        nc.tensor.matmul(
                            tp_t[:, i * 128:(i + 1) * 128],
                            lhsT=dw_t[:, nb * 128:(nb + 1) * 128],
                            rhs=rhs_op, start=True, stop=True)
                    nc.scalar.copy(
                        out=dst[:, g * 4:g * 4 + 4, cof:cof + 128],
                        in_=tp_t.rearrange("p (a b) -> p a b", a=4))

            def do_gram(g):
                # raw gram (q unnormalized; k pre-scaled); softmax as single
                # full-row ops with T*rn_q folded into the exp scale and
                # cross-head entries killed by a block-diagonal mask
                pg = psml.tile([128, 128], f32, tag="tp")
                co = g * 128
                for nb in range(32):
                    nc.tensor.matmul(
                        pg,
                        lhsT=qT_s[:, nb, co:co + 128],
                        rhs=kT_s[:, nb, co:co + 128],
                        start=(nb == 0), stop=(nb == 31),
                    )
                rnq = rnq_v[g]
                mx = small_p.tile([128, 1], f32, tag="mx")
                nc.vector.tensor_reduce(out=mx, in_=pg,
                                        axis=mybir.AxisListType.X, op=OP.max)
                ngm = small_p.tile([128, 1], f32, tag="ngm")
                nc.vector.tensor_scalar(out=ngm, in0=mx, scalar1=rnq,
                                        scalar2=-1.0, op0=OP.mult,
                                        op1=OP.mult)
                A_t = a_p.tile([128, 128], bf16, tag="A")
                nc.scalar.activation(out=A_t, in_=pg, func=AF.Exp,
                                     scale=rnq, bias=ngm)
                nc.vector.tensor_tensor(out=A_t, in0=A_t, in1=hmask_s,
                                        op=OP.mult)
                sm = small_p.tile([128, 1], f32, tag="sm")
                nc.vector.tensor_reduce(out=sm, in_=A_t,
                                        axis=mybir.AxisListType.X, op=OP.add)
                rs = small_p.tile([128, 1], f32, tag="rs")
                nc.vector.reciprocal(rs, sm)
                rs_v[g] = rs
                pa = psml.tile([128, 128], bf16, tag="tp")
                nc.tensor.transpose(pa, A_t, ident)
                At = a_p.tile([128, 128], bf16, tag="At")
                nc.scalar.copy(out=At, in_=pa)
                At_v[g] = At

            ALL_TAPS = [(0, 0), (-1, -1), (-1, 0), (-1, 1), (0, -1),
                        (0, 1), (1, -1), (1, 0), (1, 1)]

            Bts_v = [None, None]

            def do_av_prep(g):
                # dwconv folded into attention: out = sum_t shift_t(B_t @ v)
                # with B_t[d,c] = At[d,c] * w_dw[v-chan d, tap t]
                Bts = []
                for t9, (dy, dx) in enumerate(ALL_TAPS):
                    Bt = b_p.tile([128, 128], bf16, tag="Bt",
                                  name=f"B{g}_{t9}")
                    wcol = (4 + g) * 9 + (dy + 1) * 3 + (dx + 1)
                    nc.vector.tensor_scalar_mul(
                        Bt, At_v[g], wdw_s[:, wcol:wcol + 1])
                    Bts.append(Bt)
                Bts_v[g] = Bts

            def do_av_part(g, t8s):
                Bts = Bts_v[g]
                v3 = qkv_v[g].rearrange("p (y x) -> p y x", y=H)
                for t8 in t8s:
                    yt = t8 * 8
                    pv = pdw.tile([128, TS], f32, tag="pdw", name="pv")
                    pv3 = pv.rearrange("p (y x) -> p y x", y=8)
                    ops = []
                    for t9, (dy, dx) in enumerate(ALL_TAPS):
                        ya, yb = max(yt, -dy), min(yt + 8, 64 - dy)
                        xa, xb = max(0, -dx), 64 - max(0, dx)
                        if ya < yb:
                            ops.append((t9, dy, dx, ya, yb, xa, xb))
                    for j, (t9, dy, dx, ya, yb, xa, xb) in enumerate(ops):
                        nc.tensor.matmul(
                            pv3[:, ya - yt:yb - yt, xa:xb],
                            lhsT=Bts[t9],
                            rhs=v3[:, ya + dy:yb + dy, xa + dx:xb + dx],
                            start=(j == 0), stop=(j == len(ops) - 1))
                    nc.scalar.mul(attn8[:, g, yt * 64:(yt + 8) * 64],
                                  pv, rs_v[g])

            # software-pipelined emission: each engine's stream executes in
            # order, so later-phase PE work (transposes/gram/av) is emitted
            # only once enough independent PE work precedes it to cover the
            # DVE/ACT chains it waits on
            qkv_phase(2)
            tap_phase(2)
            qkv_phase(0)
            tap_phase(0)
            qkv_phase(4)
            qkv_phase(3)
            tap_phase(3)
            tp_phase(2)
            tp_phase(0)
            do_gram(0)
            qkv_phase(1)
            tap_phase(1)
            qkv_phase(5)
            tp_phase(3)
            do_av_prep(0)
            do_av_part(0, range(8))
            tp_phase(1)
            do_gram(1)
            do_av_prep(1)

            # ---- streamed tail ----
            # residuals are folded into PSUM via identity matmuls, so the
            # per-tile chain is PE -> ACT -> PE -> ACT (no DVE hops)
            x1b = [dwv_p.tile([128, N], bf16, tag="dwv", name=f"x1b{i}")
                   for i in range(2)]
            ys_t = ysp.tile([128, 3, N], f8, tag="ysf8", name="ys")
            # ones-row at hidden index 307 (kb2-local row 51): the mlp2
            # weight row there holds 16*b2, folding the bias into the matmul
            nc.vector.memset(ys_t[:, 2, :], 1.0)

            def proj_phase(t):
                sl = slice(t * 1024, (t + 1) * 1024)
                for ob in range(2):
                    pp = pbig.tile([128, 1024], f32, tag="pbig", name="pp")
                    for h in range(2):
                        nc.tensor.matmul(
                            pp[:, h * TS:(h + 1) * TS],
                            lhsT=wproj_s[:, :, ob * 128:(ob + 1) * 128],
                            rhs=attn8[:, :, t * 1024 + h * TS:
                                      t * 1024 + (h + 1) * TS],
                            perf_mode=DR, start=True, stop=True)
                    nc.vector.scalar_tensor_tensor(
                        out=x1b[ob][:, sl], in0=pp, scalar=1.0 / 16.0,
                        in1=xr[:, ob, sl], op0=OP.mult, op1=OP.add)

            def mlp1_phase(t):
                sl = slice(t * 1024, (t + 1) * 1024)
                for mb in range(3):
                    rows = 128 if mb < 2 else HID - 256
                    pm = pbig.tile([128, 1024], f32, tag="pbig", name="pm")
                    for h in range(2):
                        for kb in range(2):
                            nc.tensor.matmul(
                                pm[:rows, h * TS:(h + 1) * TS],
                                lhsT=wm1_s[:, kb, mb * 128:mb * 128 + rows],
                                rhs=x1b[kb][:, t * 1024 + h * TS:
                                            t * 1024 + (h + 1) * TS],
                                start=(kb == 0), stop=(kb == 1))
                    nc.scalar.activation(
                        out=ys_t[:rows, mb, sl],
                        in_=pm[:rows, :], func=AF.Gelu_apprx_tanh,
                        bias=b1_s[:rows, mb:mb + 1])

            def mlp2_phase(t):
                sl = slice(t * 1024, (t + 1) * 1024)
                for ob in range(2):
                    pm2 = pbig.tile([128, 1024], f32, tag="pbig", name="pm2")
                    for h in range(2):
                        hs = slice(t * 1024 + h * TS, t * 1024 + (h + 1) * TS)
                        nc.tensor.matmul(
                            pm2[:, h * TS:(h + 1) * TS],
                            lhsT=wm2_s[:, 0:2, ob * 128:(ob + 1) * 128],
                            rhs=ys_t[:, 0:2, hs],
                            perf_mode=DR, start=True, stop=False)
                        nc.tensor.matmul(
                            pm2[:, h * TS:(h + 1) * TS],
                            lhsT=wm2_s[:52, 2, ob * 128:(ob + 1) * 128],
                            rhs=ys_t[:52, 2, hs],
                            start=False, stop=True)
                    ot = out_p.tile([128, 1024], f32, tag="ot",
                                    name=f"ot{t}_{ob}")
                    nc.vector.scalar_tensor_tensor(
                        out=ot, in0=pm2,
                        scalar=1.0 / 16.0, in1=x1b[ob][:, sl],
                        op0=OP.mult, op1=OP.add)
                    nc.sync.dma_start(out=out_d[:, ob, sl], in_=ot)

            # pipelined emission: av1 chunks woven into the tail's proj
            # stream; every PE group's ACT dependency is covered by the
            # preceding PE group
            do_av_part(1, [0, 1])
            proj_phase(0)
            do_av_part(1, [2, 3])
            proj_phase(1)
            do_av_part(1, [4, 5])
            mlp1_phase(0)
            do_av_part(1, [6, 7])
            proj_phase(2)
            mlp1_phase(1)
            mlp2_phase(0)
            proj_phase(3)
            mlp1_phase(2)
            mlp2_phase(1)
            mlp1_phase(3)
            mlp2_phase(2)
            mlp2_phase(3)

    return nc


def _prep_shared(w_qkv, w_dw, temperature, w_proj, w_mlp1, b_mlp1, w_mlp2, b_mlp2):
    f32 = np.float32
    shared = {}
    F8 = ml_dtypes.float8_e4m3
    shared["wqkv8"] = np.ascontiguousarray(
        (w_qkv.T * 16.0).reshape(2, 128, 3 * C).transpose(1, 0, 2)).astype(F8)
    wd = np.zeros((128, len(PE_TAPS), 4, 128), BF16)
    for ti, (dy, dx) in enumerate(PE_TAPS):
        for cb in range(4):
            w = w_dw[cb * 128:(cb + 1) * 128, 0, dy + 1, dx + 1].astype(f32)
            wd[:, ti, cb, :] = np.diag(w).astype(BF16)
    shared["wdiag"] = wd
    wd2 = np.zeros((128, 2, 2, 128), BF16)
    for tj, (dy, dx) in enumerate(PE2_TAPS):
        for bi, cb in enumerate((1, 3)):
            w = w_dw[cb * 128:(cb + 1) * 128, 0, dy + 1, dx + 1].astype(f32)
            wd2[:, tj, bi, :] = np.diag(w).astype(BF16)
    shared["wdiag2"] = wd2
    hm = np.zeros((128, 128), f32)
    for h4 in range(4):
        hm[h4 * 32:(h4 + 1) * 32, h4 * 32:(h4 + 1) * 32] = 1.0
    shared["hmask"] = hm.astype(BF16)
    wt = np.zeros((128, NB_QKV * 9 * 2), f32)
    for cb in range(NB_QKV):
        for t in range(9):
            wt[:, cb * 9 + t] = w_dw[cb * 128:(cb + 1) * 128, 0, t // 3, t % 3]
    wt[:, 54:] = -wt[:, :54]
    shared["wdw"] = wt
    shared["wproj8"] = np.ascontiguousarray(
        (w_proj.T * 16.0).reshape(2, 128, C).transpose(1, 0, 2)).astype(F8)
    shared["wm1T"] = np.ascontiguousarray(
        w_mlp1.T.reshape(2, 128, HID).transpose(1, 0, 2)).astype(BF16)
    w2 = np.zeros((384, C), f32)
    w2[:HID] = w_mlp2.T * 16.0
    w2[307] = b_mlp2 * 16.0     # ones-row in ys folds the bias in
    shared["wm28"] = np.ascontiguousarray(
        w2.reshape(3, 128, C).transpose(1, 0, 2)).astype(F8)
    b1 = np.zeros((384,), f32)
    b1[:HID] = b_mlp1
    shared["b1"] = np.ascontiguousarray(b1.reshape(3, 128).T)
    t = temperature.reshape(NH).astype(f32)
    tv = np.zeros((128, 2), f32)
    for g in range(2):
        tv[:, g] = np.repeat(t[g * 4:(g + 1) * 4], 32)
    shared["lntv"] = np.log(np.maximum(tv, 1e-30)).astype(f32)
    return shared


def kernel(x, w_qkv, w_dw, temperature, w_proj, w_mlp1, b_mlp1, w_mlp2, b_mlp2,
           _trace=False):
    from concourse.bass_utils import run_bass_kernel_spmd

    if "nc" not in _CACHE:
        nc = _build_bass()
        nc.finalize()
        _CACHE["nc"] = nc
    nc = _CACHE["nc"]

    x = np.asarray(x, np.float32)
    B = x.shape[0]
    shared = _prep_shared(
        np.asarray(w_qkv, np.float32), np.asarray(w_dw, np.float32),
        np.asarray(temperature, np.float32), np.asarray(w_proj, np.float32),
        np.asarray(w_mlp1, np.float32), np.asarray(b_mlp1, np.float32),
        np.asarray(w_mlp2, np.float32), np.asarray(b_mlp2, np.float32))

    in_maps = []
    for i in range(B):
        m = dict(shared)
        xi = np.ascontiguousarray(x[i].reshape(2, 128, N).transpose(1, 0, 2))
        m["xb"] = xi.astype(BF16)
        m["xf8"] = xi.astype(ml_dtypes.float8_e4m3)
        in_maps.append(m)

    res = run_bass_kernel_spmd(nc, in_maps, core_ids=list(range(B)),
                               trace=_trace)
    outs = np.stack([
        r["out"].transpose(1, 0, 2).reshape(C, H, W) for r in res.results
    ])
    if _trace:
        _CACHE["last_exec_ns"] = res.exec_time_ns
        _CACHE["last_profile"] = res.profile_json
    return outs



# revision 70
# speedup vs baseline: 1.1054x; 1.0270x over previous
"""Trainium2 Bass kernel for nn_CustomABlock (MDTA transformer block).

Per-core layout: one batch image [C=256, N=4096(=64x64)] per NeuronCore,
data-parallel over B=8 across 8 cores, all params replicated.

Engine plan (per core):
  PE   : qkv matmul (f32r), 2 dwconv taps (diag matmul), q/k transposes,
         gram (attn logits), attn@v, proj, mlp1, mlp2
  DVE  : 6 dwconv taps (scalar_tensor_tensor FMA, bf16), residual adds,
         reciprocals, row-max reduces, x1 bf16 copy
  ACT  : PSUM drains, l2norm squares (accum), exp (softmax), gelu+bias
  GPSIMD: 1 dwconv tap, identity build
"""

import numpy as np
import ml_dtypes

BF16 = ml_dtypes.bfloat16

C = 256          # dim
N = 4096         # 64*64
H = W = 64
NH = 8           # heads
CH = 32          # channels per head
HID = 307        # mlp hidden
NB_QKV = 6       # qkv channel blocks of 128
NT = 8           # n tiles of 512
TS = 512

# tap index t = (dy+1)*3 + (dx+1)
PE_TAPS = [(0, 0), (-1, 0), (1, 0), (0, -1), (0, 1)]  # PE diag matmuls into PSUM
MERGE_TAP = (1, 1)                  # DVE STT: tap + PSUM drain in one op
DVE_TAPS = [(-1, -1), (-1, 1), (1, -1)]   # DVE tensor_scalar + tensor_tensor
PE2_TAPS = [(-1, -1), (1, -1)]      # extra PE corner taps for late blocks 1/3

_CACHE = {}


def _build_bass():
    import concourse.bass as bass
    from concourse import bacc
    from concourse import mybir
    from concourse.tile import TileContext
    from concourse.masks import make_identity

    # Steer the act-table-load pass: hide Exp/Ln from every set except
    # natural_log_exp_and_others so all transcendentals (norm ln/exp +
    # softmax exp) share one table load instead of ping-ponging sets.
    # Set order (= act_func_set_id) is preserved; the chosen set really
    # does contain both functions, so the loads stay correct.
    if not getattr(bacc, "_act_tables_patched", False):
        _orig_tables = bacc.get_activation_tables
        AF_ = mybir.ActivationFunctionType

        def _patched_tables(arch):
            tabs = {k: set(v) for k, v in _orig_tables(arch).items()}
            for name, fns in tabs.items():
                if name != "natural_log_exp_and_others":
                    fns.discard(AF_.Exp)
                    fns.discard(AF_.Ln)
            return tabs

        bacc.get_activation_tables = _patched_tables
        bacc._act_tables_patched = True

    dt = mybir.dt
    f32 = dt.float32
    f32r = dt.float32r
    bf16 = dt.bfloat16
    AF = mybir.ActivationFunctionType
    OP = mybir.AluOpType

    nc = bacc.Bacc("TRN2")

    f8 = dt.float8e4

    # ---- DRAM I/O (per-core) ----
    xb_d = nc.dram_tensor("xb", [128, 2, N], bf16, kind="ExternalInput")
    xf8_d = nc.dram_tensor("xf8", [128, 2, N], f8, kind="ExternalInput")
    wqkv8_d = nc.dram_tensor("wqkv8", [128, 2, 3 * C], f8, kind="ExternalInput")
    wdiag_d = nc.dram_tensor("wdiag", [128, len(PE_TAPS), 4, 128], bf16,
                             kind="ExternalInput")
    wdiag2_d = nc.dram_tensor("wdiag2", [128, 2, 2, 128], bf16,
                              kind="ExternalInput")
    hmask_d = nc.dram_tensor("hmask", [128, 128], bf16, kind="ExternalInput")
    wdw_d = nc.dram_tensor("wdw", [128, NB_QKV * 9 * 2], f32, kind="ExternalInput")
    wproj_d = nc.dram_tensor("wproj8", [128, 2, C], f8, kind="ExternalInput")
    wm1_d = nc.dram_tensor("wm1T", [128, 2, HID], bf16, kind="ExternalInput")
    wm2_d = nc.dram_tensor("wm28", [128, 3, C], f8, kind="ExternalInput")
    b1_d = nc.dram_tensor("b1", [128, 3], f32, kind="ExternalInput")
    lntv_d = nc.dram_tensor("lntv", [128, 2], f32, kind="ExternalInput")
    out_d = nc.dram_tensor("out", [128, 2, N], f32, kind="ExternalOutput")

    with TileContext(nc) as tc:
        with (
            tc.tile_pool(name="wpool", bufs=1) as wpool,
            tc.tile_pool(name="xpool", bufs=1) as xpool,
            tc.tile_pool(name="qkvp", bufs=4) as qkvp,       # qkv_s blocks / ys reuse
            tc.tile_pool(name="dwqk", bufs=4) as dwqk_p,     # dw q/k blocks
            tc.tile_pool(name="dwv", bufs=2) as dwv_p,       # x1b tiles
            tc.tile_pool(name="qt", bufs=1) as qt_p,
            tc.tile_pool(name="scr", bufs=2) as scr_p,
            tc.tile_pool(name="small", bufs=10) as small_p,
            tc.tile_pool(name="dg", bufs=2) as dg_p,
            tc.tile_pool(name="bt", bufs=18) as b_p,
            tc.tile_pool(name="attn", bufs=1) as atn_p,
            tc.tile_pool(name="ysp", bufs=1) as ysp,
            tc.tile_pool(name="outp", bufs=2) as out_p,
            tc.tile_pool(name="apool", bufs=2) as a_p,
            tc.tile_pool(name="pbig", bufs=2, space="PSUM") as pbig,
            tc.tile_pool(name="pdw", bufs=2, space="PSUM") as pdw,
            tc.tile_pool(name="psml", bufs=2, space="PSUM") as psml,
        ):
            # dummy Ln on an always-ready memset tile: makes the FIRST
            # act-table load the natural_log_exp set (which also covers
            # copy/square); emitted before any other ACT-stream work
            zz = small_p.tile([128, 1], f32, tag="zz")
            nc.vector.memset(zz, 1.0)
            dmy = small_p.tile([128, 1], f32, tag="dmy")
            nc.scalar.activation(out=dmy, in_=zz, func=AF.Ln)

            # ---- load weights & x (critical path first) ----
            xr = xpool.tile([128, 2, N], bf16)
            xf8 = xpool.tile([128, 2, N], f8)
            wqkv_s = wpool.tile([128, 2, 3 * C], f8)
            for kb in range(2):
                nc.sync.dma_start(out=wqkv_s[:, kb, :], in_=wqkv8_d[:, kb, :])
            # x fp8 chunks t-ordered so the first qkv matmuls start early;
            # split across the two HWDGE rings (SP + ACT) to halve the
            # serial startup latency
            for t in range(4):
                # both kb planes in one DMA: the DoubleRow rhs reads both,
                # so this keeps the dependency per-chunk
                nc.sync.dma_start(
                    out=xf8[:, :, t * 1024:(t + 1) * 1024],
                    in_=xf8_d[:, :, t * 1024:(t + 1) * 1024])
            wdiag_s = wpool.tile([128, len(PE_TAPS), 4, 128], bf16)
            nc.scalar.dma_start(out=wdiag_s, in_=wdiag_d[:, :, :, :])
            wdiag2_s = wpool.tile([128, 2, 2, 128], bf16)
            nc.scalar.dma_start(out=wdiag2_s, in_=wdiag2_d[:, :, :, :])
            wdw_s = wpool.tile([128, NB_QKV * 9 * 2], f32)
            nc.scalar.dma_start(out=wdw_s, in_=wdw_d[:, :])
            # xr (bf16 residual base) is only needed by the tail
            for kb in range(2):
                nc.sync.dma_start(out=xr[:, kb, :], in_=xb_d[:, kb, :])
            # tail-phase weights go via the idle GPSIMD's SWDGE ring so the
            # ACT stream isn't blocked by DMA-issue instructions
            hmask_s = wpool.tile([128, 128], bf16)
            nc.gpsimd.dma_start(out=hmask_s[:, :], in_=hmask_d[:, :])
            wproj_s = wpool.tile([128, 2, C], f8)
            nc.gpsimd.dma_start(out=wproj_s, in_=wproj_d[:, :, :])
            wm1_s = wpool.tile([128, 2, HID], bf16)
            nc.gpsimd.dma_start(out=wm1_s, in_=wm1_d[:, :, :])
            wm2_s = wpool.tile([128, 3, C], f8)
            nc.gpsimd.dma_start(out=wm2_s, in_=wm2_d[:, :, :])
            b1_s = wpool.tile([128, 3], f32)
            nc.gpsimd.dma_start(out=b1_s, in_=b1_d[:, :])
            lntv_s = wpool.tile([128, 2], f32)
            nc.scalar.dma_start(out=lntv_s, in_=lntv_d[:, :])

            ident = wpool.tile([128, 128], bf16)
            make_identity(nc, ident)

            # ---- per-block pipeline ----
            dw_tiles = [None] * NB_QKV
            qT_s = qt_p.tile([128, 32, C], bf16, tag="qT")
            kT_s = qt_p.tile([128, 32, C], bf16, tag="kT")
            attn8 = atn_p.tile([128, 2, N], f8, tag="attn")
            rs_v = [None, None]
            At_v = [None, None]
            rnq_v = [None, None]

            qkv_v = [None, None]   # v_lin tiles for the av-fold
            qkv_tiles = {}
            rhs_ops = {}

            DR = mybir.MatmulPerfMode.DoubleRow

            def qkv_phase(ob):
                # qkv = W_qkv @ x: fp8 DoubleRow folds the K=256 contraction
                # into one matmul (weights pre-scaled x16; drain undoes it)
                qkv_t = qkvp.tile([128, N], bf16, tag="qkv", name=f"qkv{ob}")
                for t in range(4):
                    ps = pbig.tile([128, 1024], f32, tag="pbig", name="ps")
                    for h in range(2):
                        nc.tensor.matmul(
                            ps[:, h * TS:(h + 1) * TS],
                            lhsT=wqkv_s[:, :, ob * 128:(ob + 1) * 128],
                            rhs=xf8[:, :, t * 1024 + h * TS:
                                    t * 1024 + (h + 1) * TS],
                            perf_mode=DR, start=True, stop=True,
                        )
                    nc.scalar.mul(qkv_t[:, t * 1024:(t + 1) * 1024], ps,
                                  1.0 / 16.0)
                qkv_tiles[ob] = qkv_t
                if ob >= 4:
                    qkv_v[ob - 4] = qkv_t

            def tap_phase(ob):
                # dwconv: PE diag taps (flat shifts) -> PSUM[128,512],
                # merge/corner taps + wrap fixups on DVE, then the l2 norm
                qkv_t = qkv_tiles[ob]
                dw_t = dwqk_p.tile([128, N], bf16, tag="dwqk", name=f"dw{ob}")
                dw_tiles[ob] = dw_t
                dw3 = dw_t.rearrange("p (y x) -> p y x", y=H)
                qk3 = qkv_t.rearrange("p (y x) -> p y x", y=H)
                dym, dxm = MERGE_TAP
                wm = wdw_s[:, ob * 9 + (dym + 1) * 3 + dxm + 1:
                           ob * 9 + (dym + 1) * 3 + dxm + 2]
                w01 = wdw_s[:, 54 + ob * 9 + 5:54 + ob * 9 + 6]
                # blocks 1/3 run late: move 2 corner taps to PE to shorten
                # their serial DVE chain (wrap-fixup columns handled below)
                pe_corner = ob in (1, 3)
                for t8 in range(8):
                    pd = pdw.tile([128, TS], f32, tag="pdw", name="pd")
                    pd3 = pd.rearrange("p (y x) -> p y x", y=8)
                    c0 = t8 * TS
                    ops = []
                    for ti, (dy, dx) in enumerate(PE_TAPS):
                        s = dy * 64 + dx
                        a = max(c0, -s)
                        b = min(c0 + TS, N - max(0, s))
                        if a < b:
                            ops.append(((0, ti), s, a, b))
                    if pe_corner:
                        for tj, (dy, dx) in enumerate(PE2_TAPS):
                            s = dy * 64 + dx
                            a = max(c0, -s)
                            b = min(c0 + TS, N - max(0, s))
                            if a < b:
                                ops.append(((1, tj), s, a, b))
                    for j, ((bank, ti), s, a, b) in enumerate(ops):
                        lhs = (wdiag_s[:, ti, ob, :] if bank == 0 else
                               wdiag2_s[:, ti, (ob - 1) // 2, :])
                        nc.tensor.matmul(
                            pd[:, a - c0:b - c0],
                            lhsT=lhs,
                            rhs=qkv_t[:, a + s:b + s],
                            start=(j == 0), stop=(j == len(ops) - 1),
                        )
                    yt = t8 * 8
                    # merge tap (1,1): dw = w*qkv[y+1,x+1] + psum (drains)
                    ya, yb = yt, min(yt + 8, 63)
                    nc.vector.scalar_tensor_tensor(
                        out=dw3[:, ya:yb, 0:63],
                        in0=qk3[:, ya + 1:yb + 1, 1:64],
                        scalar=wm,
                        in1=pd3[:, 0:yb - yt, 0:63],
                        op0=OP.mult, op1=OP.add,
                    )
                    # x=63 col: drain PSUM minus tap(0,1) row-wrap
                    nc.vector.scalar_tensor_tensor(
                        out=dw3[:, yt:yb, 63:64],
                        in0=qk3[:, yt + 1:yb + 1, 0:1],
                        scalar=w01, in1=pd3[:, 0:yb - yt, 63:64],
                        op0=OP.mult, op1=OP.add,
                    )
                    if t8 == 7:
                        nc.scalar.copy(out=dw3[:, 63:64, :],
                                       in_=pd3[:, 7:8, :])
                # x=0 col: subtract tap(0,-1) row-wrap (whole block, in place)
                w0m = wdw_s[:, 54 + ob * 9 + 3:54 + ob * 9 + 4]
                nc.vector.scalar_tensor_tensor(
                    out=dw3[:, 1:64, 0:1], in0=qk3[:, 0:63, 63:64],
                    scalar=w0m, in1=dw3[:, 1:64, 0:1],
                    op0=OP.mult, op1=OP.add,
                )

                if pe_corner:
                    # wrap fixups for the PE corner taps (subtract the
                    # spurious x-wrap column contributions)
                    # (-1,-1) s=-65: out(y,0) wrongly read (y-2,63)
                    wn = wdw_s[:, 54 + ob * 9 + 0:54 + ob * 9 + 1]
                    nc.vector.scalar_tensor_tensor(
                        out=dw3[:, 2:64, 0:1], in0=qk3[:, 0:62, 63:64],
                        scalar=wn, in1=dw3[:, 2:64, 0:1],
                        op0=OP.mult, op1=OP.add)
                    # (1,-1) s=+63: out(y,0) wrongly read (y,63)
                    wn6 = wdw_s[:, 54 + ob * 9 + 6:54 + ob * 9 + 7]
                    nc.vector.scalar_tensor_tensor(
                        out=dw3[:, 0:64, 0:1], in0=qk3[:, 0:64, 63:64],
                        scalar=wn6, in1=dw3[:, 0:64, 0:1],
                        op0=OP.mult, op1=OP.add)
                # remaining corner taps: tensor_scalar (4x) + tensor_tensor
                for (dy, dx) in ([(-1, 1)] if pe_corner else DVE_TAPS):
                    ti = (dy + 1) * 3 + (dx + 1)
                    w_ap = wdw_s[:, ob * 9 + ti:ob * 9 + ti + 1]
                    y0, y1 = max(0, -dy), 64 - max(0, dy)
                    x0, x1 = max(0, -dx), 64 - max(0, dx)
                    sc_t = scr_p.tile([128, N], bf16, tag="sqscr",
                                      name=f"scr{ob}_{ti}")
                    sc3 = sc_t.rearrange("p (y x) -> p y x", y=H)
                    nc.vector.tensor_scalar_mul(
                        sc3[:, y0:y1, x0:x1],
                        qk3[:, y0 + dy:y1 + dy, x0 + dx:x1 + dx], w_ap)
                    nc.vector.tensor_tensor(
                        out=dw3[:, y0:y1, x0:x1], in0=dw3[:, y0:y1, x0:x1],
                        in1=sc3[:, y0:y1, x0:x1], op=OP.add)

                # l2 norm: ssq -> rn = exp(-0.5*ln(ssq) [+ ln(T)]), all in
                # the natural_log_exp ACT table set
                sq = scr_p.tile([128, N], bf16, tag="sqscr")
                ssq = small_p.tile([128, 1], f32, tag="ssq")
                nc.scalar.activation(out=sq, in_=dw_t, func=AF.Square,
                                     accum_out=ssq)
                lg = small_p.tile([128, 1], f32, tag="lg")
                nc.scalar.activation(out=lg, in_=ssq, func=AF.Ln)
                if ob < 2:
                    # q: T/|q| applied later as the softmax-exp scale — the
                    # transposes below don't wait on the norm chain
                    rn = small_p.tile([128, 1], f32, tag=f"rnq{ob}")
                    nc.scalar.activation(out=rn, in_=lg, func=AF.Exp,
                                         scale=-0.5,
                                         bias=lntv_s[:, ob:ob + 1])
                    rnq_v[ob] = rn
                    rhs_ops[ob] = ident
                else:
                    # k: scale must be in kT before the gram — fold into the
                    # transpose matmul via D = diag(rn)
                    rn = small_p.tile([128, 1], f32, tag="rn")
                    nc.scalar.activation(out=rn, in_=lg, func=AF.Exp,
                                         scale=-0.5)
                    D_t = dg_p.tile([128, 128], bf16, tag="D")
                    nc.vector.tensor_scalar_mul(D_t, ident, rn)
                    rhs_ops[ob] = D_t

            def tp_phase(ob):
                dw_t = dw_tiles[ob]
                rhs_op = rhs_ops[ob]
                dst = qT_s if ob < 2 else kT_s
                cof = (ob % 2) * 128
                for g in range(8):
                    tp_t = psml.tile([128, 512], f32, tag="tp")
                    for i in range(4):
                        nb = g * 4 + i
                        # regular matmul: out = dw_chunk.T @ rhs_op — a
                        # transpose that (for k) applies the row scale
                        # (transpose-mode ignores the operand values)
                        nc.tensor.matmul(
                            tp_t[:, i * 128:(i + 1) * 128],
                            lhsT=dw_t[:, nb * 128:(nb + 1) * 128],
                            rhs=rhs_op, start=True, stop=True)
                    nc.scalar.copy(
                        out=dst[:, g * 4:g * 4 + 4, cof:cof + 128],
                        in_=tp_t.rearrange("p (a b) -> p a b", a=4))

            def do_gram(g):
                # raw gram (q unnormalized; k pre-scaled); softmax as single
                # full-row ops with T*rn_q folded into the exp scale and
                # cross-head entries killed by a block-diagonal mask
                pg = psml.tile([128, 128], f32, tag="tp")
                co = g * 128
                for nb in range(32):
                    nc.tensor.matmul(
                        pg,
                        lhsT=qT_s[:, nb, co:co + 128],
                        rhs=kT_s[:, nb, co:co + 128],
                        start=(nb == 0), stop=(nb == 31),
                    )
                rnq = rnq_v[g]
                mx = small_p.tile([128, 1], f32, tag="mx")
                nc.vector.tensor_reduce(out=mx, in_=pg,
                                        axis=mybir.AxisListType.X, op=OP.max)
                ngm = small_p.tile([128, 1], f32, tag="ngm")
                nc.vector.tensor_scalar(out=ngm, in0=mx, scalar1=rnq,
                                        scalar2=-1.0, op0=OP.mult,
                                        op1=OP.mult)
                A_t = a_p.tile([128, 128], bf16, tag="A")
                nc.scalar.activation(out=A_t, in_=pg, func=AF.Exp,
                                     scale=rnq, bias=ngm)
                nc.vector.tensor_tensor(out=A_t, in0=A_t, in1=hmask_s,
                                        op=OP.mult)
                sm = small_p.tile([128, 1], f32, tag="sm")
                nc.vector.tensor_reduce(out=sm, in_=A_t,
                                        axis=mybir.AxisListType.X, op=OP.add)
                rs = small_p.tile([128, 1], f32, tag="rs")
                nc.vector.reciprocal(rs, sm)
                rs_v[g] = rs
                pa = psml.tile([128, 128], bf16, tag="tp")
                nc.tensor.transpose(pa, A_t, ident)
                At = a_p.tile([128, 128], bf16, tag="At")
                nc.scalar.copy(out=At, in_=pa)
                At_v[g] = At

            ALL_TAPS = [(0, 0), (-1, -1), (-1, 0), (-1, 1), (0, -1),
                        (0, 1), (1, -1), (1, 0), (1, 1)]

            Bts_v = [None, None]

            def do_av_prep(g):
                # dwconv folded into attention: out = sum_t shift_t(B_t @ v)
                # with B_t[d,c] = At[d,c] * w_dw[v-chan d, tap t]
                Bts = []
                for t9, (dy, dx) in enumerate(ALL_TAPS):
                    Bt = b_p.tile([128, 128], bf16, tag="Bt",
                                  name=f"B{g}_{t9}")
                    wcol = (4 + g) * 9 + (dy + 1) * 3 + (dx + 1)
                    nc.vector.tensor_scalar_mul(
                        Bt, At_v[g], wdw_s[:, wcol:wcol + 1])
                    Bts.append(Bt)
                Bts_v[g] = Bts

            def do_av_part(g, t8s):
                Bts = Bts_v[g]
                v3 = qkv_v[g].rearrange("p (y x) -> p y x", y=H)
                for t8 in t8s:
                    yt = t8 * 8
                    pv = pdw.tile([128, TS], f32, tag="pdw", name="pv")
                    pv3 = pv.rearrange("p (y x) -> p y x", y=8)
                    ops = []
                    for t9, (dy, dx) in enumerate(ALL_TAPS):
                        ya, yb = max(yt, -dy), min(yt + 8, 64 - dy)
                        xa, xb = max(0, -dx), 64 - max(0, dx)
                        if ya < yb:
                            ops.append((t9, dy, dx, ya, yb, xa, xb))
                    for j, (t9, dy, dx, ya, yb, xa, xb) in enumerate(ops):
                        nc.tensor.matmul(
                            pv3[:, ya - yt:yb - yt, xa:xb],
                            lhsT=Bts[t9],
                            rhs=v3[:, ya + dy:yb + dy, xa + dx:xb + dx],
                            start=(j == 0), stop=(j == len(ops) - 1))
                    nc.scalar.mul(attn8[:, g, yt * 64:(yt + 8) * 64],
                                  pv, rs_v[g])

            # software-pipelined emission: each engine's stream executes in
            # order, so later-phase PE work (transposes/gram/av) is emitted
            # only once enough independent PE work precedes it to cover the
            # DVE/ACT chains it waits on
            qkv_phase(2)
            tap_phase(2)
            qkv_phase(0)
            tap_phase(0)
            qkv_phase(4)
            qkv_phase(3)
            tap_phase(3)
            tp_phase(2)
            tp_phase(0)
            do_gram(0)
            qkv_phase(1)
            tap_phase(1)
            qkv_phase(5)
            tp_phase(3)
            do_av_prep(0)
            do_av_part(0, [0, 1, 2, 3])
            tp_phase(1)
            do_av_part(0, [4, 5])
            do_gram(1)
            do_av_part(0, [6, 7])
            do_av_prep(1)

            # ---- streamed tail ----
            # residuals are folded into PSUM via identity matmuls, so the
            # per-tile chain is PE -> ACT -> PE -> ACT (no DVE hops)
            x1b = [dwv_p.tile([128, N], bf16, tag="dwv", name=f"x1b{i}")
                   for i in range(2)]
            ys_t = ysp.tile([128, 3, N], f8, tag="ysf8", name="ys")
            # ones-row at hidden index 307 (kb2-local row 51): the mlp2
            # weight row there holds 16*b2, folding the bias into the matmul
            nc.vector.memset(ys_t[:, 2, :], 1.0)

            def proj_phase(t):
                sl = slice(t * 1024, (t + 1) * 1024)
                for ob in range(2):
                    pp = pbig.tile([128, 1024], f32, tag="pbig", name="pp")
                    for h in range(2):
                        nc.tensor.matmul(
                            pp[:, h * TS:(h + 1) * TS],
                            lhsT=wproj_s[:, :, ob * 128:(ob + 1) * 128],
                            rhs=attn8[:, :, t * 1024 + h * TS:
                                      t * 1024 + (h + 1) * TS],
                            perf_mode=DR, start=True, stop=True)
                    nc.vector.scalar_tensor_tensor(
                        out=x1b[ob][:, sl], in0=pp, scalar=1.0 / 16.0,
                        in1=xr[:, ob, sl], op0=OP.mult, op1=OP.add)

            def mlp1_phase(t):
                sl = slice(t * 1024, (t + 1) * 1024)
                for mb in range(3):
                    rows = 128 if mb < 2 else HID - 256
                    pm = pbig.tile([128, 1024], f32, tag="pbig", name="pm")
                    for h in range(2):
                        for kb in range(2):
                            nc.tensor.matmul(
                                pm[:rows, h * TS:(h + 1) * TS],
                                lhsT=wm1_s[:, kb, mb * 128:mb * 128 + rows],
                                rhs=x1b[kb][:, t * 1024 + h * TS:
                                            t * 1024 + (h + 1) * TS],
                                start=(kb == 0), stop=(kb == 1))
                    nc.scalar.activation(
                        out=ys_t[:rows, mb, sl],
                        in_=pm[:rows, :], func=AF.Gelu_apprx_tanh,
                        bias=b1_s[:rows, mb:mb + 1])

            def mlp2_phase(t):
                sl = slice(t * 1024, (t + 1) * 1024)
                for ob in range(2):
                    pm2 = pbig.tile([128, 1024], f32, tag="pbig", name="pm2")
                    for h in range(2):
                        hs = slice(t * 1024 + h * TS, t * 1024 + (h + 1) * TS)
                        nc.tensor.matmul(
                            pm2[:, h * TS:(h + 1) * TS],
                            lhsT=wm2_s[:, 0:2, ob * 128:(ob + 1) * 128],
                            rhs=ys_t[:, 0:2, hs],
                            perf_mode=DR, start=True, stop=False)
                        nc.tensor.matmul(
                            pm2[:, h * TS:(h + 1) * TS],
                            lhsT=wm2_s[:52, 2, ob * 128:(ob + 1) * 128],
                            rhs=ys_t[:52, 2, hs],
                            start=False, stop=True)
                    ot = out_p.tile([128, 1024], f32, tag="ot",
                                    name=f"ot{t}_{ob}")
                    nc.vector.scalar_tensor_tensor(
                        out=ot, in0=pm2,
                        scalar=1.0 / 16.0, in1=x1b[ob][:, sl],
                        op0=OP.mult, op1=OP.add)
                    nc.sync.dma_start(out=out_d[:, ob, sl], in_=ot)

            # pipelined emission: av1 chunks woven into the tail's proj
            # stream; every PE group's ACT dependency is covered by the
            # preceding PE group
            do_av_part(1, [0, 1])
            proj_phase(0)
            do_av_part(1, [2, 3])
            proj_phase(1)
            do_av_part(1, [4, 5])
            mlp1_phase(0)
            do_av_part(1, [6, 7])
            proj_phase(2)
            mlp1_phase(1)
            mlp2_phase(0)
            proj_phase(3)
            mlp1_phase(2)
            mlp2_phase(1)
            mlp1_phase(3)
            mlp2_phase(2)
            mlp2_phase(3)

    return nc


def _prep_shared(w_qkv, w_dw, temperature, w_proj, w_mlp1, b_mlp1, w_mlp2, b_mlp2):
    f32 = np.float32
    shared = {}
    F8 = ml_dtypes.float8_e4m3
    shared["wqkv8"] = np.ascontiguousarray(
        (w_qkv.T * 16.0).reshape(2, 128, 3 * C).transpose(1, 0, 2)).astype(F8)
    wd = np.zeros((128, len(PE_TAPS), 4, 128), BF16)
    for ti, (dy, dx) in enumerate(PE_TAPS):
        for cb in range(4):
            w = w_dw[cb * 128:(cb + 1) * 128, 0, dy + 1, dx + 1].astype(f32)
            wd[:, ti, cb, :] = np.diag(w).astype(BF16)
    shared["wdiag"] = wd
    wd2 = np.zeros((128, 2, 2, 128), BF16)
    for tj, (dy, dx) in enumerate(PE2_TAPS):
        for bi, cb in enumerate((1, 3)):
            w = w_dw[cb * 128:(cb + 1) * 128, 0, dy + 1, dx + 1].astype(f32)
            wd2[:, tj, bi, :] = np.diag(w).astype(BF16)
    shared["wdiag2"] = wd2
    hm = np.zeros((128, 128), f32)
    for h4 in range(4):
        hm[h4 * 32:(h4 + 1) * 32, h4 * 32:(h4 + 1) * 32] = 1.0
    shared["hmask"] = hm.astype(BF16)
    wt = np.zeros((128, NB_QKV * 9 * 2), f32)
    for cb in range(NB_QKV):
        for t in range(9):
            wt[:, cb * 9 + t] = w_dw[cb * 128:(cb + 1) * 128, 0, t // 3, t % 3]
    wt[:, 54:] = -wt[:, :54]
    shared["wdw"] = wt
    shared["wproj8"] = np.ascontiguousarray(
        (w_proj.T * 16.0).reshape(2, 128, C).transpose(1, 0, 2)).astype(F8)
    shared["wm1T"] = np.ascontiguousarray(
        w_mlp1.T.reshape(2, 128, HID).transpose(1, 0, 2)).astype(BF16)
    w2 = np.zeros((384, C), f32)
    w2[:HID] = w_mlp2.T * 16.0
    w2[307] = b_mlp2 * 16.0     # ones-row in ys folds the bias in
    shared["wm28"] = np.ascontiguousarray(
        w2.reshape(3, 128, C).transpose(1, 0, 2)).astype(F8)
    b1 = np.zeros((384,), f32)
    b1[:HID] = b_mlp1
    shared["b1"] = np.ascontiguousarray(b1.reshape(3, 128).T)
    t = temperature.reshape(NH).astype(f32)
    tv = np.zeros((128, 2), f32)
    for g in range(2):
        tv[:, g] = np.repeat(t[g * 4:(g + 1) * 4], 32)
    shared["lntv"] = np.log(np.maximum(tv, 1e-30)).astype(f32)
    return shared


def kernel(x, w_qkv, w_dw, temperature, w_proj, w_mlp1, b_mlp1, w_mlp2, b_mlp2,
           _trace=False):
    from concourse.bass_utils import run_bass_kernel_spmd

    if "nc" not in _CACHE:
        nc = _build_bass()
        nc.finalize()
        _CACHE["nc"] = nc
    nc = _CACHE["nc"]

    x = np.asarray(x, np.float32)
    B = x.shape[0]
    shared = _prep_shared(
        np.asarray(w_qkv, np.float32), np.asarray(w_dw, np.float32),
        np.asarray(temperature, np.float32), np.asarray(w_proj, np.float32),
        np.asarray(w_mlp1, np.float32), np.asarray(b_mlp1, np.float32),
        np.asarray(w_mlp2, np.float32), np.asarray(b_mlp2, np.float32))

    in_maps = []
    for i in range(B):
        m = dict(shared)
        xi = np.ascontiguousarray(x[i].reshape(2, 128, N).transpose(1, 0, 2))
        m["xb"] = xi.astype(BF16)
        m["xf8"] = xi.astype(ml_dtypes.float8_e4m3)
        in_maps.append(m)

    res = run_bass_kernel_spmd(nc, in_maps, core_ids=list(range(B)),
                               trace=_trace)
    outs = np.stack([
        r["out"].transpose(1, 0, 2).reshape(C, H, W) for r in res.results
    ])
    if _trace:
        _CACHE["last_exec_ns"] = res.exec_time_ns
        _CACHE["last_profile"] = res.profile_json
    return outs



# revision 73
# speedup vs baseline: 1.1464x; 1.0371x over previous
"""Trainium2 Bass kernel for nn_CustomABlock (MDTA transformer block).

Per-core layout: one batch image [C=256, N=4096(=64x64)] per NeuronCore,
data-parallel over B=8 across 8 cores, all params replicated.

Engine plan (per core):
  PE   : qkv matmul (f32r), 2 dwconv taps (diag matmul), q/k transposes,
         gram (attn logits), attn@v, proj, mlp1, mlp2
  DVE  : 6 dwconv taps (scalar_tensor_tensor FMA, bf16), residual adds,
         reciprocals, row-max reduces, x1 bf16 copy
  ACT  : PSUM drains, l2norm squares (accum), exp (softmax), gelu+bias
  GPSIMD: 1 dwconv tap, identity build
"""

import numpy as np
import ml_dtypes

BF16 = ml_dtypes.bfloat16

C = 256          # dim
N = 4096         # 64*64
H = W = 64
NH = 8           # heads
CH = 32          # channels per head
HID = 307        # mlp hidden
NB_QKV = 6       # qkv channel blocks of 128
NT = 8           # n tiles of 512
TS = 512

# tap index t = (dy+1)*3 + (dx+1)
PE_TAPS = [(0, 0), (-1, 0), (1, 0), (0, -1), (0, 1)]  # PE diag matmuls into PSUM
MERGE_TAP = (1, 1)                  # DVE STT: tap + PSUM drain in one op
DVE_TAPS = [(-1, -1), (-1, 1), (1, -1)]   # DVE tensor_scalar + tensor_tensor
PE2_TAPS = [(-1, -1), (1, -1)]      # extra PE corner taps for late blocks 1/3

_CACHE = {}


def _build_bass():
    import concourse.bass as bass
    from concourse import bacc
    from concourse import mybir
    from concourse.tile import TileContext
    from concourse.masks import make_identity

    # Steer the act-table-load pass: hide Exp/Ln from every set except
    # natural_log_exp_and_others so all transcendentals (norm ln/exp +
    # softmax exp) share one table load instead of ping-ponging sets.
    # Set order (= act_func_set_id) is preserved; the chosen set really
    # does contain both functions, so the loads stay correct.
    if not getattr(bacc, "_act_tables_patched", False):
        _orig_tables = bacc.get_activation_tables
        AF_ = mybir.ActivationFunctionType

        def _patched_tables(arch):
            tabs = {k: set(v) for k, v in _orig_tables(arch).items()}
            for name, fns in tabs.items():
                if name != "natural_log_exp_and_others":
                    fns.discard(AF_.Exp)
                    fns.discard(AF_.Ln)
            return tabs

        bacc.get_activation_tables = _patched_tables
        bacc._act_tables_patched = True

    dt = mybir.dt
    f32 = dt.float32
    f32r = dt.float32r
    bf16 = dt.bfloat16
    AF = mybir.ActivationFunctionType
    OP = mybir.AluOpType

    nc = bacc.Bacc("TRN2")

    f8 = dt.float8e4

    # ---- DRAM I/O (per-core) ----
    xb_d = nc.dram_tensor("xb", [128, 2, N], bf16, kind="ExternalInput")
    xf8_d = nc.dram_tensor("xf8", [128, 2, N], f8, kind="ExternalInput")
    wqkv8_d = nc.dram_tensor("wqkv8", [128, 2, 3 * C], f8, kind="ExternalInput")
    wdiag_d = nc.dram_tensor("wdiag", [128, len(PE_TAPS), 4, 128], bf16,
                             kind="ExternalInput")
    wdiag2_d = nc.dram_tensor("wdiag2", [128, 2, 2, 128], bf16,
                              kind="ExternalInput")
    hmask_d = nc.dram_tensor("hmask", [128, 128], bf16, kind="ExternalInput")
    wdw_d = nc.dram_tensor("wdw", [128, NB_QKV * 9 * 2], f32, kind="ExternalInput")
    wproj_d = nc.dram_tensor("wproj8", [128, 2, C], f8, kind="ExternalInput")
    wm1_d = nc.dram_tensor("wm1T", [128, 2, HID], bf16, kind="ExternalInput")
    wm2_d = nc.dram_tensor("wm28", [128, 3, C], f8, kind="ExternalInput")
    b1_d = nc.dram_tensor("b1", [128, 3], f32, kind="ExternalInput")
    lntv_d = nc.dram_tensor("lntv", [128, 2], f32, kind="ExternalInput")
    out_d = nc.dram_tensor("out", [128, 2, N], f32, kind="ExternalOutput")

    with TileContext(nc) as tc:
        with (
            tc.tile_pool(name="wpool", bufs=1) as wpool,
            tc.tile_pool(name="xpool", bufs=1) as xpool,
            tc.tile_pool(name="qkvp", bufs=4) as qkvp,       # qkv_s blocks / ys reuse
            tc.tile_pool(name="dwqk", bufs=4) as dwqk_p,     # dw q/k blocks
            tc.tile_pool(name="dwv", bufs=2) as dwv_p,       # x1b tiles
            tc.tile_pool(name="qt", bufs=1) as qt_p,
            tc.tile_pool(name="scr", bufs=2) as scr_p,
            tc.tile_pool(name="small", bufs=10) as small_p,
            tc.tile_pool(name="dg", bufs=2) as dg_p,
            tc.tile_pool(name="bt", bufs=18) as b_p,
            tc.tile_pool(name="attn", bufs=1) as atn_p,
            tc.tile_pool(name="ysp", bufs=1) as ysp,
            tc.tile_pool(name="outp", bufs=2) as out_p,
            tc.tile_pool(name="apool", bufs=2) as a_p,
            tc.tile_pool(name="pbig", bufs=2, space="PSUM") as pbig,
            tc.tile_pool(name="pdw", bufs=2, space="PSUM") as pdw,
            tc.tile_pool(name="psml", bufs=2, space="PSUM") as psml,
        ):
            # dummy Ln on an always-ready memset tile: makes the FIRST
            # act-table load the natural_log_exp set (which also covers
            # copy/square); emitted before any other ACT-stream work
            zz = small_p.tile([128, 1], f32, tag="zz")
            nc.vector.memset(zz, 1.0)
            dmy = small_p.tile([128, 1], f32, tag="dmy")
            nc.scalar.activation(out=dmy, in_=zz, func=AF.Ln)

            # ---- load weights & x (critical path first) ----
            xr = xpool.tile([128, 2, N], bf16)
            xf8 = xpool.tile([128, 2, N], f8)
            wqkv_s = wpool.tile([128, 2, 3 * C], f8)
            for kb in range(2):
                nc.sync.dma_start(out=wqkv_s[:, kb, :], in_=wqkv8_d[:, kb, :])
            # x fp8 chunks t-ordered so the first qkv matmuls start early;
            # split across the two HWDGE rings (SP + ACT) to halve the
            # serial startup latency
            for t in range(4):
                # both kb planes in one DMA: the DoubleRow rhs reads both,
                # so this keeps the dependency per-chunk
                nc.sync.dma_start(
                    out=xf8[:, :, t * 1024:(t + 1) * 1024],
                    in_=xf8_d[:, :, t * 1024:(t + 1) * 1024])
            wdiag_s = wpool.tile([128, len(PE_TAPS), 4, 128], bf16)
            nc.scalar.dma_start(out=wdiag_s, in_=wdiag_d[:, :, :, :])
            wdiag2_s = wpool.tile([128, 2, 2, 128], bf16)
            nc.scalar.dma_start(out=wdiag2_s, in_=wdiag2_d[:, :, :, :])
            wdw_s = wpool.tile([128, NB_QKV * 9 * 2], f32)
            nc.scalar.dma_start(out=wdw_s, in_=wdw_d[:, :])
            # xr (bf16 residual base) is only needed by the tail
            for kb in range(2):
                nc.sync.dma_start(out=xr[:, kb, :], in_=xb_d[:, kb, :])
            # tail-phase weights go via the idle GPSIMD's SWDGE ring so the
            # ACT stream isn't blocked by DMA-issue instructions
            hmask_s = wpool.tile([128, 128], bf16)
            nc.gpsimd.dma_start(out=hmask_s[:, :], in_=hmask_d[:, :])
            wproj_s = wpool.tile([128, 2, C], f8)
            nc.gpsimd.dma_start(out=wproj_s, in_=wproj_d[:, :, :])
            wm1_s = wpool.tile([128, 2, HID], bf16)
            nc.gpsimd.dma_start(out=wm1_s, in_=wm1_d[:, :, :])
            wm2_s = wpool.tile([128, 3, C], f8)
            nc.gpsimd.dma_start(out=wm2_s, in_=wm2_d[:, :, :])
            b1_s = wpool.tile([128, 3], f32)
            nc.gpsimd.dma_start(out=b1_s, in_=b1_d[:, :])
            lntv_s = wpool.tile([128, 2], f32)
            nc.scalar.dma_start(out=lntv_s, in_=lntv_d[:, :])

            ident = wpool.tile([128, 128], bf16)
            make_identity(nc, ident)

            # ---- per-block pipeline ----
            dw_tiles = [None] * NB_QKV
            qT_s = qt_p.tile([128, 32, C], bf16, tag="qT")
            kT_s = qt_p.tile([128, 32, C], bf16, tag="kT")
            attn8 = atn_p.tile([128, 2, N], f8, tag="attn")
            rs_v = [None, None]
            At_v = [None, None]
            rnq_v = [None, None]

            qkv_v = [None, None]   # v_lin tiles for the av-fold
            qkv_tiles = {}
            rhs_ops = {}

            DR = mybir.MatmulPerfMode.DoubleRow

            def qkv_phase(ob):
                # qkv = W_qkv @ x: fp8 DoubleRow folds the K=256 contraction
                # into one matmul (weights pre-scaled x16; drain undoes it)
                qkv_t = qkvp.tile([128, N], bf16, tag="qkv", name=f"qkv{ob}")
                for t in range(4):
                    ps = pbig.tile([128, 1024], f32, tag="pbig", name="ps")
                    for h in range(2):
                        nc.tensor.matmul(
                            ps[:, h * TS:(h + 1) * TS],
                            lhsT=wqkv_s[:, :, ob * 128:(ob + 1) * 128],
                            rhs=xf8[:, :, t * 1024 + h * TS:
                                    t * 1024 + (h + 1) * TS],
                            perf_mode=DR, start=True, stop=True,
                        )
                    nc.scalar.mul(qkv_t[:, t * 1024:(t + 1) * 1024], ps,
                                  1.0 / 16.0)
                qkv_tiles[ob] = qkv_t
                if ob >= 4:
                    qkv_v[ob - 4] = qkv_t

            def tap_phase(ob):
                # dwconv: PE diag taps (flat shifts) -> PSUM[128,512],
                # merge/corner taps + wrap fixups on DVE, then the l2 norm
                qkv_t = qkv_tiles[ob]
                dw_t = dwqk_p.tile([128, N], bf16, tag="dwqk", name=f"dw{ob}")
                dw_tiles[ob] = dw_t
                dw3 = dw_t.rearrange("p (y x) -> p y x", y=H)
                qk3 = qkv_t.rearrange("p (y x) -> p y x", y=H)
                dym, dxm = MERGE_TAP
                wm = wdw_s[:, ob * 9 + (dym + 1) * 3 + dxm + 1:
                           ob * 9 + (dym + 1) * 3 + dxm + 2]
                w01 = wdw_s[:, 54 + ob * 9 + 5:54 + ob * 9 + 6]
                # blocks 1/3 run late: move 2 corner taps to PE to shorten
                # their serial DVE chain (wrap-fixup columns handled below)
                pe_corner = ob in (1, 3)
                for t8 in range(8):
                    pd = pdw.tile([128, TS], f32, tag="pdw", name="pd")
                    pd3 = pd.rearrange("p (y x) -> p y x", y=8)
                    c0 = t8 * TS
                    ops = []
                    for ti, (dy, dx) in enumerate(PE_TAPS):
                        s = dy * 64 + dx
                        a = max(c0, -s)
                        b = min(c0 + TS, N - max(0, s))
                        if a < b:
                            ops.append(((0, ti), s, a, b))
                    if pe_corner:
                        for tj, (dy, dx) in enumerate(PE2_TAPS):
                            s = dy * 64 + dx
                            a = max(c0, -s)
                            b = min(c0 + TS, N - max(0, s))
                            if a < b:
                                ops.append(((1, tj), s, a, b))
                    for j, ((bank, ti), s, a, b) in enumerate(ops):
                        lhs = (wdiag_s[:, ti, ob, :] if bank == 0 else
                               wdiag2_s[:, ti, (ob - 1) // 2, :])
                        nc.tensor.matmul(
                            pd[:, a - c0:b - c0],
                            lhsT=lhs,
                            rhs=qkv_t[:, a + s:b + s],
                            start=(j == 0), stop=(j == len(ops) - 1),
                        )
                    yt = t8 * 8
                    # merge tap (1,1): dw = w*qkv[y+1,x+1] + psum (drains)
                    ya, yb = yt, min(yt + 8, 63)
                    nc.vector.scalar_tensor_tensor(
                        out=dw3[:, ya:yb, 0:63],
                        in0=qk3[:, ya + 1:yb + 1, 1:64],
                        scalar=wm,
                        in1=pd3[:, 0:yb - yt, 0:63],
                        op0=OP.mult, op1=OP.add,
                    )
                    # x=63 col: drain PSUM minus tap(0,1) row-wrap
                    nc.vector.scalar_tensor_tensor(
                        out=dw3[:, yt:yb, 63:64],
                        in0=qk3[:, yt + 1:yb + 1, 0:1],
                        scalar=w01, in1=pd3[:, 0:yb - yt, 63:64],
                        op0=OP.mult, op1=OP.add,
                    )
                    if t8 == 7:
                        nc.scalar.copy(out=dw3[:, 63:64, :],
                                       in_=pd3[:, 7:8, :])
                # x=0 col: subtract tap(0,-1) row-wrap (whole block, in place)
                w0m = wdw_s[:, 54 + ob * 9 + 3:54 + ob * 9 + 4]
                nc.vector.scalar_tensor_tensor(
                    out=dw3[:, 1:64, 0:1], in0=qk3[:, 0:63, 63:64],
                    scalar=w0m, in1=dw3[:, 1:64, 0:1],
                    op0=OP.mult, op1=OP.add,
                )

                if pe_corner:
                    # wrap fixups for the PE corner taps (subtract the
                    # spurious x-wrap column contributions)
                    # (-1,-1) s=-65: out(y,0) wrongly read (y-2,63)
                    wn = wdw_s[:, 54 + ob * 9 + 0:54 + ob * 9 + 1]
                    nc.vector.scalar_tensor_tensor(
                        out=dw3[:, 2:64, 0:1], in0=qk3[:, 0:62, 63:64],
                        scalar=wn, in1=dw3[:, 2:64, 0:1],
                        op0=OP.mult, op1=OP.add)
                    # (1,-1) s=+63: out(y,0) wrongly read (y,63)
                    wn6 = wdw_s[:, 54 + ob * 9 + 6:54 + ob * 9 + 7]
                    nc.vector.scalar_tensor_tensor(
                        out=dw3[:, 0:64, 0:1], in0=qk3[:, 0:64, 63:64],
                        scalar=wn6, in1=dw3[:, 0:64, 0:1],
                        op0=OP.mult, op1=OP.add)
                # remaining corner taps: tensor_scalar (4x) + tensor_tensor
                for (dy, dx) in ([(-1, 1)] if pe_corner else DVE_TAPS):
                    ti = (dy + 1) * 3 + (dx + 1)
                    w_ap = wdw_s[:, ob * 9 + ti:ob * 9 + ti + 1]
                    y0, y1 = max(0, -dy), 64 - max(0, dy)
                    x0, x1 = max(0, -dx), 64 - max(0, dx)
                    sc_t = scr_p.tile([128, N], bf16, tag="sqscr",
                                      name=f"scr{ob}_{ti}")
                    sc3 = sc_t.rearrange("p (y x) -> p y x", y=H)
                    nc.vector.tensor_scalar_mul(
                        sc3[:, y0:y1, x0:x1],
                        qk3[:, y0 + dy:y1 + dy, x0 + dx:x1 + dx], w_ap)
                    nc.vector.tensor_tensor(
                        out=dw3[:, y0:y1, x0:x1], in0=dw3[:, y0:y1, x0:x1],
                        in1=sc3[:, y0:y1, x0:x1], op=OP.add)

                if ob < 2:
                    # q: the norm is only needed at softmax time — emitted
                    # later (norm_phase) so it doesn't block transpose drains
                    rhs_ops[ob] = ident
                else:
                    # k: scale must be in kT before the gram — fold into the
                    # transpose matmul via D = diag(rn)
                    ssq = _ssq_norm(ob, dw_t)
                    lg = small_p.tile([128, 1], f32, tag="lg")
                    nc.scalar.activation(out=lg, in_=ssq, func=AF.Ln)
                    rn = small_p.tile([128, 1], f32, tag="rn")
                    nc.scalar.activation(out=rn, in_=lg, func=AF.Exp,
                                         scale=-0.5)
                    D_t = dg_p.tile([128, 128], bf16, tag="D")
                    nc.vector.tensor_scalar_mul(D_t, ident, rn)
                    rhs_ops[ob] = D_t

            def _ssq_norm(ob, dw_t):
                # l2 norm sum-of-squares via ACT Square with accumulate
                sq = scr_p.tile([128, N], bf16, tag="sqscr")
                ssq = small_p.tile([128, 1], f32, tag="ssq")
                nc.scalar.activation(out=sq, in_=dw_t, func=AF.Square,
                                     accum_out=ssq)
                return ssq

            def norm_phase(ob):
                # q norm: rn_q = exp(-0.5*ln(ssq) + ln(T)), consumed as the
                # softmax-exp scale in do_gram
                ssq = _ssq_norm(ob, dw_tiles[ob])
                lg = small_p.tile([128, 1], f32, tag="lg")
                nc.scalar.activation(out=lg, in_=ssq, func=AF.Ln)
                rn = small_p.tile([128, 1], f32, tag=f"rnq{ob}")
                nc.scalar.activation(out=rn, in_=lg, func=AF.Exp,
                                     scale=-0.5, bias=lntv_s[:, ob:ob + 1])
                rnq_v[ob] = rn

            def tp_phase(ob):
                dw_t = dw_tiles[ob]
                rhs_op = rhs_ops[ob]
                dst = qT_s if ob < 2 else kT_s
                cof = (ob % 2) * 128
                for g in range(8):
                    tp_t = psml.tile([128, 512], f32, tag="tp")
                    for i in range(4):
                        nb = g * 4 + i
                        # regular matmul: out = dw_chunk.T @ rhs_op — a
                        # transpose that (for k) applies the row scale
                        # (transpose-mode ignores the operand values)
                        nc.tensor.matmul(
                            tp_t[:, i * 128:(i + 1) * 128],
                            lhsT=dw_t[:, nb * 128:(nb + 1) * 128],
                            rhs=rhs_op, start=True, stop=True)
                    nc.scalar.copy(
                        out=dst[:, g * 4:g * 4 + 4, cof:cof + 128],
                        in_=tp_t.rearrange("p (a b) -> p a b", a=4))

            def do_gram(g):
                # raw gram (q unnormalized; k pre-scaled); softmax as single
                # full-row ops with T*rn_q folded into the exp scale and
                # cross-head entries killed by a block-diagonal mask
                pg = psml.tile([128, 128], f32, tag="tp")
                co = g * 128
                for nb in range(32):
                    nc.tensor.matmul(
                        pg,
                        lhsT=qT_s[:, nb, co:co + 128],
                        rhs=kT_s[:, nb, co:co + 128],
                        start=(nb == 0), stop=(nb == 31),
                    )
                rnq = rnq_v[g]
                mx = small_p.tile([128, 1], f32, tag="mx")
                nc.vector.tensor_reduce(out=mx, in_=pg,
                                        axis=mybir.AxisListType.X, op=OP.max)
                ngm = small_p.tile([128, 1], f32, tag="ngm")
                nc.vector.tensor_scalar(out=ngm, in0=mx, scalar1=rnq,
                                        scalar2=-1.0, op0=OP.mult,
                                        op1=OP.mult)
                A_t = a_p.tile([128, 128], bf16, tag="A")
                nc.scalar.activation(out=A_t, in_=pg, func=AF.Exp,
                                     scale=rnq, bias=ngm)
                nc.vector.tensor_tensor(out=A_t, in0=A_t, in1=hmask_s,
                                        op=OP.mult)
                sm = small_p.tile([128, 1], f32, tag="sm")
                nc.vector.tensor_reduce(out=sm, in_=A_t,
                                        axis=mybir.AxisListType.X, op=OP.add)
                rs = small_p.tile([128, 1], f32, tag="rs")
                nc.vector.reciprocal(rs, sm)
                rs_v[g] = rs
                pa = psml.tile([128, 128], bf16, tag="tp")
                nc.tensor.transpose(pa, A_t, ident)
                At = a_p.tile([128, 128], bf16, tag="At")
                nc.scalar.copy(out=At, in_=pa)
                At_v[g] = At

            ALL_TAPS = [(0, 0), (-1, -1), (-1, 0), (-1, 1), (0, -1),
                        (0, 1), (1, -1), (1, 0), (1, 1)]

            Bts_v = [None, None]

            def do_av_prep(g):
                # dwconv folded into attention: out = sum_t shift_t(B_t @ v)
                # with B_t[d,c] = At[d,c] * w_dw[v-chan d, tap t]
                Bts = []
                for t9, (dy, dx) in enumerate(ALL_TAPS):
                    Bt = b_p.tile([128, 128], bf16, tag="Bt",
                                  name=f"B{g}_{t9}")
                    wcol = (4 + g) * 9 + (dy + 1) * 3 + (dx + 1)
                    nc.vector.tensor_scalar_mul(
                        Bt, At_v[g], wdw_s[:, wcol:wcol + 1])
                    Bts.append(Bt)
                Bts_v[g] = Bts

            def do_av_part(g, t8s):
                Bts = Bts_v[g]
                v3 = qkv_v[g].rearrange("p (y x) -> p y x", y=H)
                for t8 in t8s:
                    yt = t8 * 8
                    pv = pdw.tile([128, TS], f32, tag="pdw", name="pv")
                    pv3 = pv.rearrange("p (y x) -> p y x", y=8)
                    ops = []
                    for t9, (dy, dx) in enumerate(ALL_TAPS):
                        ya, yb = max(yt, -dy), min(yt + 8, 64 - dy)
                        xa, xb = max(0, -dx), 64 - max(0, dx)
                        if ya < yb:
                            ops.append((t9, dy, dx, ya, yb, xa, xb))
                    for j, (t9, dy, dx, ya, yb, xa, xb) in enumerate(ops):
                        nc.tensor.matmul(
                            pv3[:, ya - yt:yb - yt, xa:xb],
                            lhsT=Bts[t9],
                            rhs=v3[:, ya + dy:yb + dy, xa + dx:xb + dx],
                            start=(j == 0), stop=(j == len(ops) - 1))
                    nc.scalar.mul(attn8[:, g, yt * 64:(yt + 8) * 64],
                                  pv, rs_v[g])

            # software-pipelined emission: each engine's stream executes in
            # order, so later-phase PE work (transposes/gram/av) is emitted
            # only once enough independent PE work precedes it to cover the
            # DVE/ACT chains it waits on
            qkv_phase(2)
            tap_phase(2)
            qkv_phase(0)
            tap_phase(0)
            qkv_phase(4)
            qkv_phase(3)
            tap_phase(3)
            tp_phase(2)
            tp_phase(0)
            norm_phase(0)
            do_gram(0)
            qkv_phase(1)
            tap_phase(1)
            qkv_phase(5)
            tp_phase(3)
            do_av_prep(0)
            do_av_part(0, [0, 1, 2, 3])
            tp_phase(1)
            norm_phase(1)
            do_av_part(0, [4, 5])
            do_gram(1)
            do_av_part(0, [6, 7])
            do_av_prep(1)

            # ---- streamed tail ----
            # residuals are folded into PSUM via identity matmuls, so the
            # per-tile chain is PE -> ACT -> PE -> ACT (no DVE hops)
            x1b = [dwv_p.tile([128, N], bf16, tag="dwv", name=f"x1b{i}")
                   for i in range(2)]
            ys_t = ysp.tile([128, 3, N], f8, tag="ysf8", name="ys")
            # ones-row at hidden index 307 (kb2-local row 51): the mlp2
            # weight row there holds 16*b2, folding the bias into the matmul
            nc.vector.memset(ys_t[:, 2, :], 1.0)

            def proj_phase(t):
                sl = slice(t * 1024, (t + 1) * 1024)
                for ob in range(2):
                    pp = pbig.tile([128, 1024], f32, tag="pbig", name="pp")
                    for h in range(2):
                        nc.tensor.matmul(
                            pp[:, h * TS:(h + 1) * TS],
                            lhsT=wproj_s[:, :, ob * 128:(ob + 1) * 128],
                            rhs=attn8[:, :, t * 1024 + h * TS:
                                      t * 1024 + (h + 1) * TS],
                            perf_mode=DR, start=True, stop=True)
                    nc.vector.scalar_tensor_tensor(
                        out=x1b[ob][:, sl], in0=pp, scalar=1.0 / 16.0,
                        in1=xr[:, ob, sl], op0=OP.mult, op1=OP.add)

            def mlp1_phase(t):
                sl = slice(t * 1024, (t + 1) * 1024)
                for mb in range(3):
                    rows = 128 if mb < 2 else HID - 256
                    pm = pbig.tile([128, 1024], f32, tag="pbig", name="pm")
                    for h in range(2):
                        for kb in range(2):
                            nc.tensor.matmul(
                                pm[:rows, h * TS:(h + 1) * TS],
                                lhsT=wm1_s[:, kb, mb * 128:mb * 128 + rows],
                                rhs=x1b[kb][:, t * 1024 + h * TS:
                                            t * 1024 + (h + 1) * TS],
                                start=(kb == 0), stop=(kb == 1))
                    nc.scalar.activation(
                        out=ys_t[:rows, mb, sl],
                        in_=pm[:rows, :], func=AF.Gelu_apprx_tanh,
                        bias=b1_s[:rows, mb:mb + 1])

            def mlp2_phase(t):
                sl = slice(t * 1024, (t + 1) * 1024)
                for ob in range(2):
                    pm2 = pbig.tile([128, 1024], f32, tag="pbig", name="pm2")
                    for h in range(2):
                        hs = slice(t * 1024 + h * TS, t * 1024 + (h + 1) * TS)
                        nc.tensor.matmul(
                            pm2[:, h * TS:(h + 1) * TS],
                            lhsT=wm2_s[:, 0:2, ob * 128:(ob + 1) * 128],
                            rhs=ys_t[:, 0:2, hs],
                            perf_mode=DR, start=True, stop=False)
                        nc.tensor.matmul(
                            pm2[:, h * TS:(h + 1) * TS],
                            lhsT=wm2_s[:52, 2, ob * 128:(ob + 1) * 128],
                            rhs=ys_t[:52, 2, hs],
                            start=False, stop=True)
                    ot = out_p.tile([128, 1024], f32, tag="ot",
                                    name=f"ot{t}_{ob}")
                    nc.vector.scalar_tensor_tensor(
                        out=ot, in0=pm2,
                        scalar=1.0 / 16.0, in1=x1b[ob][:, sl],
                        op0=OP.mult, op1=OP.add)
                    nc.sync.dma_start(out=out_d[:, ob, sl], in_=ot)

            # pipelined emission: av1 chunks woven into the tail's proj
            # stream; every PE group's ACT dependency is covered by the
            # preceding PE group
            do_av_part(1, [0, 1])
            proj_phase(0)
            do_av_part(1, [2, 3])
            proj_phase(1)
            do_av_part(1, [4, 5])
            mlp1_phase(0)
            do_av_part(1, [6, 7])
            proj_phase(2)
            mlp1_phase(1)
            mlp2_phase(0)
            proj_phase(3)
            mlp1_phase(2)
            mlp2_phase(1)
            mlp1_phase(3)
            mlp2_phase(2)
            mlp2_phase(3)

    return nc


def _prep_shared(w_qkv, w_dw, temperature, w_proj, w_mlp1, b_mlp1, w_mlp2, b_mlp2):
    f32 = np.float32
    shared = {}
    F8 = ml_dtypes.float8_e4m3
    shared["wqkv8"] = np.ascontiguousarray(
        (w_qkv.T * 16.0).reshape(2, 128, 3 * C).transpose(1, 0, 2)).astype(F8)
    wd = np.zeros((128, len(PE_TAPS), 4, 128), BF16)
    for ti, (dy, dx) in enumerate(PE_TAPS):
        for cb in range(4):
            w = w_dw[cb * 128:(cb + 1) * 128, 0, dy + 1, dx + 1].astype(f32)
            wd[:, ti, cb, :] = np.diag(w).astype(BF16)
    shared["wdiag"] = wd
    wd2 = np.zeros((128, 2, 2, 128), BF16)
    for tj, (dy, dx) in enumerate(PE2_TAPS):
        for bi, cb in enumerate((1, 3)):
            w = w_dw[cb * 128:(cb + 1) * 128, 0, dy + 1, dx + 1].astype(f32)
            wd2[:, tj, bi, :] = np.diag(w).astype(BF16)
    shared["wdiag2"] = wd2
    hm = np.zeros((128, 128), f32)
    for h4 in range(4):
        hm[h4 * 32:(h4 + 1) * 32, h4 * 32:(h4 + 1) * 32] = 1.0
    shared["hmask"] = hm.astype(BF16)
    wt = np.zeros((128, NB_QKV * 9 * 2), f32)
    for cb in range(NB_QKV):
        for t in range(9):
            wt[:, cb * 9 + t] = w_dw[cb * 128:(cb + 1) * 128, 0, t // 3, t % 3]
    wt[:, 54:] = -wt[:, :54]
    shared["wdw"] = wt
    shared["wproj8"] = np.ascontiguousarray(
        (w_proj.T * 16.0).reshape(2, 128, C).transpose(1, 0, 2)).astype(F8)
    shared["wm1T"] = np.ascontiguousarray(
        w_mlp1.T.reshape(2, 128, HID).transpose(1, 0, 2)).astype(BF16)
    w2 = np.zeros((384, C), f32)
    w2[:HID] = w_mlp2.T * 16.0
    w2[307] = b_mlp2 * 16.0     # ones-row in ys folds the bias in
    shared["wm28"] = np.ascontiguousarray(
        w2.reshape(3, 128, C).transpose(1, 0, 2)).astype(F8)
    b1 = np.zeros((384,), f32)
    b1[:HID] = b_mlp1
    shared["b1"] = np.ascontiguousarray(b1.reshape(3, 128).T)
    t = temperature.reshape(NH).astype(f32)
    tv = np.zeros((128, 2), f32)
    for g in range(2):
        tv[:, g] = np.repeat(t[g * 4:(g + 1) * 4], 32)
    shared["lntv"] = np.log(np.maximum(tv, 1e-30)).astype(f32)
    return shared


def kernel(x, w_qkv, w_dw, temperature, w_proj, w_mlp1, b_mlp1, w_mlp2, b_mlp2,
           _trace=False):
    from concourse.bass_utils import run_bass_kernel_spmd

    if "nc" not in _CACHE:
        nc = _build_bass()
        nc.finalize()
        _CACHE["nc"] = nc
    nc = _CACHE["nc"]

    x = np.asarray(x, np.float32)
    B = x.shape[0]
    shared = _prep_shared(
        np.asarray(w_qkv, np.float32), np.asarray(w_dw, np.float32),
        np.asarray(temperature, np.float32), np.asarray(w_proj, np.float32),
        np.asarray(w_mlp1, np.float32), np.asarray(b_mlp1, np.float32),
        np.asarray(w_mlp2, np.float32), np.asarray(b_mlp2, np.float32))

    in_maps = []
    for i in range(B):
        m = dict(shared)
        xi = np.ascontiguousarray(x[i].reshape(2, 128, N).transpose(1, 0, 2))
        m["xb"] = xi.astype(BF16)
        m["xf8"] = xi.astype(ml_dtypes.float8_e4m3)
        in_maps.append(m)

    res = run_bass_kernel_spmd(nc, in_maps, core_ids=list(range(B)),
                               trace=_trace)
    outs = np.stack([
        r["out"].transpose(1, 0, 2).reshape(C, H, W) for r in res.results
    ])
    if _trace:
        _CACHE["last_exec_ns"] = res.exec_time_ns
        _CACHE["last_profile"] = res.profile_json
    return outs

